# revision 28
# baseline (speedup 1.0000x reference)
"""2-layer GAT (PyG GATConv style) distributed across 8 TRN2 NeuronCores.

Sharding (per hint): nodes partitioned into 8 contiguous destination blocks.
Per core:
  A. xp1 = x_blk @ W1 and per-node attention logits for its own block are
     packed into a per-node gather-row table (bf16, 768B pitch,
     row = [xp1 (256, (c,h)-interleaved) | als1 (4)]); dst-side logits go to
     a 256B-pitch side table. The row table is AllGathered so every core
     holds all N rows.
  C. layer-1 edge phase over the core's own destination block. Edges are
     bucketed by 128-wide destination windows ("groups") and, within a
     group, into bins of Q=4 slots sharing one destination, laid out on
     (partition, bin-tile) cells -- so dst-side data broadcasts along the
     free dim with unit-stride APs. Source rows are fetched with batched
     dma_gather (int16 indices, table split in two <32768-row halves);
     per-edge softmax numerators ex = exp(leaky_relu(als[s]+ald[d])) are
     computed on DVE/ACT, and messages are segment-summed by 0/1 selector
     matmuls on the TensorEngine (PSUM accumulation per group). The
     softmax normalizer s rides as 4 extra rhs columns of the same matmul,
     so alpha normalization is a per-node epilogue. ELU and the layer-2
     node table are fused into the epilogue; that table is AllGathered.
  F. layer-2 edge phase (same structure, 17-wide rows) + log_softmax.

Host preprocessing is index-only (self-loops, bucketing, padding,
relayout); all float math runs on device.
"""

import math
from contextlib import ExitStack

import numpy as np
import ml_dtypes

import concourse.bass as bass
import concourse.tile as tile
from concourse import bacc, mybir
from concourse.bass_utils import run_bass_kernel_spmd
from concourse.masks import make_identity

F32 = mybir.dt.float32
BF16 = mybir.dt.bfloat16
F8 = mybir.dt.float8e4
I16 = mybir.dt.int16
AF = mybir.ActivationFunctionType
OP = mybir.AluOpType

P = 128
Q = 4                 # slots per bin (same-dst edges per partition-cell)
NEG_SLOPE = 0.2


_GQ = {"i": 0}


def _next_q(nq=4):
    q = _GQ["i"] % nq
    _GQ["i"] += 1
    return q


def dma_gather_raw(gp, out_ap, in_ap, idxs_ap, num_idxs, elem_size,
                   elem_step, single_packet=None, queue_num=None):
    """BassGpSimd.dma_gather minus the payload%256 assert (pitch must still
    be a 256B multiple; verified on HW that arbitrary payload works).

    num_idxs must stay <= ~2032: the per-queue SWDGE FIFO holds 128
    entries and a call consumes num_idxs/16+1. Rotating queue_num lets
    transfers from consecutive calls overlap (one outstanding per queue).
    """
    from concourse._compat import exact_div
    if single_packet is None:
        # single-packet mode breaks somewhere between 1024 and 1536 indices
        single_packet = num_idxs <= 1024
    if queue_num is None:
        queue_num = _next_q()
    assert num_idxs <= 2032, num_idxs
    assert idxs_ap.dtype == mybir.dt.int16
    assert in_ap.dtype == out_ap.dtype
    stride_bytes = elem_step * mybir.dt.size(in_ap.dtype)
    stride_bytes_256 = exact_div(stride_bytes, 256)
    assert stride_bytes_256 < 256
    _in_ap = gp.lower_ap_dma(in_ap, for_custom_bir_dma=True)
    _idxs_ap = gp.lower_ap(idxs_ap)
    _out_ap = gp.lower_ap(out_ap)
    return gp.add_instruction(
        mybir.InstDMAGatherAnt(
            name=gp.bass.get_next_instruction_name(),
            ins=[*_in_ap, _idxs_ap, gp.lower_val_access(gp.to_reg(num_idxs))],
            outs=[_out_ap],
            transpose=False,
            num_idxs=num_idxs,
            elem_size=elem_size,
            stride_bytes_256=stride_bytes_256,
            gen_mode=0,
            single_packet=single_packet,
            queue_num=queue_num,
            sbuf_tokens_per_rank=0,
            sbuf_free_dim_per_rank=0,
            sbuf_free_dim_pad_per_rank=0,
            sbuf_byte_offset=0,
        ))


class Dims:
    def __init__(self, N, E, n_cores, H1=4, C1=64, H2=1, C2=16, F_in=256):
        self.N, self.E, self.NC = N, E, n_cores
        self.F_in = F_in
        self.H1, self.C1, self.H2, self.C2 = H1, C1, H2, C2
        self.D1 = H1 * C1
        self.D2 = H2 * C2
        self.B = N // n_cores
        self.G = math.ceil(self.B / P)
        self.HALF = min(25000, (N + 1) // 2)  # int16 table split point
        self.R1 = self.D1 + self.H1          # gathered row 1: xp1|als1
        self.T1 = 384                         # table-1 pitch (768B bf16)
        self.R2 = self.D2 + self.H2          # gathered row 2: xp2|als2
        self.T2 = 128                         # table-2 pitch (256B bf16)
        self.TA = 128                         # ald side-table pitch (256B)
        self.bA = None   # bin-tiles for src-half A (set by host_prep)
        self.bB = None
        self.KB = None   # bA + bB
        self.COLS = None  # KB * Q edge columns per group


def _wrap_idx16(flat):
    """index list -> [128, ceil(n/16)] int16 SBUF image (16-partition wrap,
    replicated for the 8 Q7 cores)."""
    n = len(flat)
    S = math.ceil(n / 16)
    a = np.zeros((16, S), np.int16)
    i = np.arange(n)
    a[i % 16, i // 16] = flat
    return np.tile(a, (8, 1))


def host_prep(dims: Dims, edge_index: np.ndarray):
    """Index-only preprocessing: self-loops, per-core dst blocks, 128-dst
    windows, same-dst bins of Q slots split by src half, padding, int16
    index images."""
    N, NC, B, G = dims.N, dims.NC, dims.B, dims.G
    HALF = dims.HALF
    loops = np.arange(N, dtype=np.int64)
    src = np.concatenate([edge_index[0].astype(np.int64), loops])
    dst = np.concatenate([edge_index[1].astype(np.int64), loops])

    # per (core, group, half): list of (dst_local, [srcs]) bins
    cores = []
    bmaxA = bmaxB = 1
    for k in range(NC):
        lo, hi = k * B, (k + 1) * B
        m = (dst >= lo) & (dst < hi)
        s_k = src[m]
        d_k = dst[m] - lo
        order = np.lexsort((s_k >= HALF, d_k))
        s_k, d_k = s_k[order], d_k[order]
        half_k = (s_k >= HALF).astype(np.int64)
        groups = []
        for g in range(G):
            gmask = (d_k // P) == g
            sg, dg, hg = s_k[gmask], d_k[gmask] - g * P, half_k[gmask]
            binsA, binsB = [], []
            for h, bins in ((0, binsA), (1, binsB)):
                hm = hg == h
                sh, dh = sg[hm], dg[hm]
                # consecutive same-dst runs -> bins of <= Q edges
                ptr = 0
                n = len(sh)
                while ptr < n:
                    d0 = dh[ptr]
                    end = ptr
                    while end < n and dh[end] == d0 and end - ptr < Q:
                        end += 1
                    bins.append((int(d0), sh[ptr:end]))
                    ptr = end
            groups.append((binsA, binsB))
            bmaxA = max(bmaxA, math.ceil(len(binsA) / P))
            bmaxB = max(bmaxB, math.ceil(len(binsB) / P))
        cores.append(groups)
    dims.bA, dims.bB = bmaxA, bmaxB
    dims.KB = bmaxA + bmaxB
    dims.COLS = dims.KB * Q

    G_, KB, COLS = G, dims.KB, dims.COLS
    per_core = []
    for k in range(NC):
        # slot-level arrays
        srcl = np.zeros((P, G_, COLS), np.int64)   # half-local src id
        emask = np.full((P, G_, COLS), -150.0, np.float32)  # pad-slot mask
        dstl = np.full((P, G_, KB), -1.0, np.float32)  # per-bin local dst
        binid = np.zeros((P, G_, KB), np.int64)    # per-bin LOCAL dst row id
        for g in range(G_):
            binsA, binsB = cores[k][g]
            for hoff, bins in ((0, binsA), (dims.bA, binsB)):
                for b, (d0, ss) in enumerate(bins):
                    p = b % P
                    bt = hoff + b // P
                    dstl[p, g, bt] = float(d0)
                    binid[p, g, bt] = g * P + d0
                    for q, s in enumerate(ss):
                        srcl[p, g, bt * Q + q] = s if s < HALF else s - HALF
                        emask[p, g, bt * Q + q] = 0.0
        # int16 gather-index images per group, concatenated along free dim
        SA = dims.bA * Q * P // 16            # idxA image cols per group
        SB = dims.bB * Q * P // 16
        SBN = KB * P // 16
        idxA = np.zeros((P, G_ * SA), np.int16)
        idxB = np.zeros((P, G_ * SB), np.int16)
        idxN = np.zeros((P, G_ * SBN), np.int16)
        for g in range(G_):
            cA = dims.bA * Q
            flatA = srcl[:, g, 0:cA].T.reshape(-1)      # i = c*128 + p
            flatB = srcl[:, g, cA:COLS].T.reshape(-1)
            flatN = binid[:, g, :].T.reshape(-1)
            idxA[:, g * SA:(g + 1) * SA] = _wrap_idx16(flatA)
            idxB[:, g * SB:(g + 1) * SB] = _wrap_idx16(flatB)
            idxN[:, g * SBN:(g + 1) * SBN] = _wrap_idx16(flatN)
        # selector one-hots: stsel[p, (g,bt,d)] = 1 iff dstl[p,g,bt] == d
        stsel = np.zeros((P, G_, KB, P), ml_dtypes.bfloat16)
        pp, gg, bb = np.nonzero(dstl >= 0)
        stsel[pp, gg, bb, dstl[pp, gg, bb].astype(np.int64)] = 1.0
        per_core.append(dict(
            idxA=idxA, idxB=idxB, idxN=idxN,
            stsel=np.ascontiguousarray(stsel.reshape(P, G_ * KB * P)),
            emask=np.ascontiguousarray(
                emask.reshape(P, G_ * COLS)).astype(ml_dtypes.bfloat16),
        ))
    return per_core


def build_program(dims: Dims, replicas: int = 1, stages: str = "agchf"):
    N, NC, B, G = dims.N, dims.NC, dims.B, dims.G
    F_in, D1, D2, H1, H2 = dims.F_in, dims.D1, dims.D2, dims.H1, dims.H2
    C1 = dims.C1
    R1, T1, R2, T2, TA = dims.R1, dims.T1, dims.R2, dims.T2, dims.TA
    bAt, bBt, KB, COLS = dims.bA, dims.bB, dims.KB, dims.COLS
    KF = F_in // P
    KD = D1 // P
    SA = bAt * Q * P // 16
    SB = bBt * Q * P // 16
    SBN = KB * P // 16
    nA = bAt * Q * P     # indices per gather call A
    nB = bBt * Q * P
    nN = KB * P
    HALF = dims.HALF

    _GQ["i"] = 0
    nc = bacc.Bacc("TRN2", target_bir_lowering=False, debug=False,
                   enable_asserts=False, num_devices=NC,
                   num_swdge_queues=4)

    xT = nc.dram_tensor("xT", [F_in, B], BF16, kind="ExternalInput")
    W1 = nc.dram_tensor("W1", [F_in, D1], BF16, kind="ExternalInput")
    a1s = nc.dram_tensor("a1s", [D1], F32, kind="ExternalInput")
    a1d = nc.dram_tensor("a1d", [D1], F32, kind="ExternalInput")
    b1 = nc.dram_tensor("b1", [D1], F32, kind="ExternalInput")
    W2 = nc.dram_tensor("W2", [D1, D2], BF16, kind="ExternalInput")
    a2s = nc.dram_tensor("a2s", [D2], F32, kind="ExternalInput")
    a2d = nc.dram_tensor("a2d", [D2], F32, kind="ExternalInput")
    b2 = nc.dram_tensor("b2", [D2], F32, kind="ExternalInput")
    idxA = nc.dram_tensor("idxA", [P, G * SA], I16, kind="ExternalInput")
    idxB = nc.dram_tensor("idxB", [P, G * SB], I16, kind="ExternalInput")
    idxN = nc.dram_tensor("idxN", [P, G * SBN], I16, kind="ExternalInput")
    stsel = nc.dram_tensor("stsel", [P, G * KB * P], BF16,
                           kind="ExternalInput")
    emask = nc.dram_tensor("emask", [P, G * COLS], BF16, kind="ExternalInput")
    out2 = nc.dram_tensor("out2", [B, D2], F32, kind="ExternalOutput")

    t1_loc = nc.dram_tensor("t1_loc", [B, T1], BF16)
    t1_full = nc.dram_tensor("t1_full", [N, T1], BF16, addr_space="Shared")
    ald1t = nc.dram_tensor("ald1t", [B, TA], BF16)
    t2_loc = nc.dram_tensor("t2_loc", [B, T2], BF16)
    t2_full = nc.dram_tensor("t2_full", [N, T2], BF16, addr_space="Shared")

    rg = [list(range(NC))]

    with tile.TileContext(nc) as tc, ExitStack() as ctx:
        const = ctx.enter_context(tc.tile_pool(name="const", bufs=1))
        ictx = ExitStack()
        cpsum = ictx.enter_context(tc.tile_pool(name="cpsum", bufs=1,
                                                space="PSUM"))

        ident = const.tile([P, P], BF16, tag="ident")
        make_identity(nc, ident[:])

        w1sb = const.tile([P, KF, D1], BF16, tag="w1sb")
        for c in range(KF):
            nc.sync.dma_start(out=w1sb[:, c, :], in_=W1[c * P:(c + 1) * P, :])
        w2sb = const.tile([P, KD, D2], BF16, tag="w2sb")
        for c in range(KD):
            nc.sync.dma_start(out=w2sb[:, c, :], in_=W2[c * P:(c + 1) * P, :])

        ones_row = const.tile([1, P], F32, tag="ones_row")
        nc.vector.memset(ones_row[:], 1.0)

        def replicate(vec_ap, X, tag):
            vrow = const.tile([1, X], F32, tag=tag + "_row")
            nc.sync.dma_start(out=vrow[:], in_=vec_ap[None, :])
            pr = cpsum.tile([P, X], F32, tag="reppsum")
            nc.tensor.matmul(out=pr[:], lhsT=ones_row[:], rhs=vrow[:],
                             start=True, stop=True)
            rep = const.tile([P, X], F32, tag=tag)
            nc.vector.tensor_copy(rep[:], pr[:])
            return rep

        a1s_r = replicate(a1s, D1, "a1s_r")
        a1d_r = replicate(a1d, D1, "a1d_r")
        b1_r = replicate(b1, D1, "b1_r")
        a2s_r = replicate(a2s, D2, "a2s_r")
        a2d_r = replicate(a2d, D2, "a2d_r")
        b2_r = replicate(b2, D2, "b2_r")

        idxA_sb = const.tile([P, G * SA], I16, tag="idxA_sb")
        nc.sync.dma_start(out=idxA_sb[:], in_=idxA[:, :])
        idxB_sb = const.tile([P, G * SB], I16, tag="idxB_sb")
        nc.sync.dma_start(out=idxB_sb[:], in_=idxB[:, :])
        idxN_sb = const.tile([P, G * SBN], I16, tag="idxN_sb")
        nc.sync.dma_start(out=idxN_sb[:], in_=idxN[:, :])
        emask_sb = const.tile([P, G * COLS], BF16, tag="emask_sb")
        nc.sync.dma_start(out=emask_sb[:], in_=emask[:, :])

        ictx.close()

        for _rep in range(replicas):
          if "a" in stages:
            # ---- stage A: layer-1 node table for own block -----------------
            actx = ExitStack()
            pa = actx.enter_context(tc.tile_pool(name="pa", bufs=3))
            pa_ps = actx.enter_context(tc.tile_pool(name="pa_ps", bufs=2,
                                                    space="PSUM"))
            for t in range(G):
                n0 = t * P
                nn = min(P, B - n0)
                xta = pa.tile([P, KF, P], BF16, tag="xta")
                for c in range(KF):
                    nc.sync.dma_start(out=xta[:, c, :nn],
                                      in_=xT[c * P:(c + 1) * P, n0:n0 + nn])
                ps_xp = pa_ps.tile([P, D1], F32, tag="ps_xp")
                for c in range(KF):
                    nc.tensor.matmul(out=ps_xp[:nn, :], lhsT=xta[:, c, :nn],
                                     rhs=w1sb[:, c, :],
                                     start=(c == 0), stop=(c == KF - 1))
                tmp = pa.tile([P, D1], F32, tag="tmpa")
                alf = pa.tile([P, 2 * H1], F32, tag="alf")
                row = pa.tile([P, R1], BF16, tag="row1")
                rowd = pa.tile([P, H1], BF16, tag="rowd")
                nc.vector.tensor_tensor(out=tmp[:nn], in0=ps_xp[:nn],
                                        in1=a1s_r[:nn], op=OP.mult)
                # features are (c,h)-interleaved: head h at stride-4 positions
                nc.vector.tensor_reduce(
                    out=alf[:nn, 0:H1],
                    in_=tmp[:nn].rearrange("p (c h) -> p h c", h=H1),
                    axis=mybir.AxisListType.X, op=OP.add)
                nc.vector.tensor_tensor(out=tmp[:nn], in0=ps_xp[:nn],
                                        in1=a1d_r[:nn], op=OP.mult)
                nc.vector.tensor_reduce(
                    out=alf[:nn, H1:2 * H1],
                    in_=tmp[:nn].rearrange("p (c h) -> p h c", h=H1),
                    axis=mybir.AxisListType.X, op=OP.add)
                nc.vector.tensor_copy(row[:nn, D1:D1 + H1], alf[:nn, 0:H1])
                nc.vector.tensor_copy(row[:nn, 0:D1], ps_xp[:nn])
                nc.vector.tensor_copy(rowd[:nn, :], alf[:nn, H1:2 * H1])
                nc.sync.dma_start(out=t1_loc[n0:n0 + nn, 0:R1], in_=row[:nn, :])
                nc.sync.dma_start(out=ald1t[n0:n0 + nn, 0:H1], in_=rowd[:nn, :])
            actx.close()

          if "g" in stages:
            # ---- AllGather layer-1 table -----------------------------------
            nc.gpsimd.collective_compute(
                "AllGather", OP.bypass, replica_groups=rg,
                ins=[t1_loc.ap()], outs=[t1_full.ap()])

          if "c" in stages:
            # ---- stage C: layer-1 edge phase + fused layer-2 table ---------
            cctx = ExitStack()
            pal = cctx.enter_context(tc.tile_pool(name="pal", bufs=6))
            pg = cctx.enter_context(tc.tile_pool(name="pg", bufs=4))
            pm = cctx.enter_context(tc.tile_pool(name="pm", bufs=3))
            pe = cctx.enter_context(tc.tile_pool(name="pe", bufs=3))
            pc_ps = cctx.enter_context(tc.tile_pool(name="pc_ps", bufs=2,
                                                    space="PSUM"))
            pt_ps = cctx.enter_context(tc.tile_pool(name="pt_ps", bufs=2,
                                                    space="PSUM"))
            cA = bAt * Q
            cB = bBt * Q
            # per-bin dst logits, two groups per call (<=2032-idx FIFO cap)
            for b0 in range(0, G, 2):
              gz = min(2, G - b0)
              aldbp = pal.tile([P, 2 * KB, H1], BF16, tag="aldbp")
              dma_gather_raw(nc.gpsimd, aldbp[:, 0:gz * KB, :],
                             ald1t[:, 0:H1],
                             idxN_sb[:, b0 * SBN:(b0 + gz) * SBN],
                             gz * nN, H1, TA)
              for g in range(b0, b0 + gz):
                w0 = g * P
                wn = min(P, B - w0)
                gA = pg.tile([P, cA, R1], BF16, tag="gatA")
                dma_gather_raw(nc.gpsimd, gA[:, :, :],
                               t1_full[0:HALF, 0:R1],
                               idxA_sb[:, g * SA:(g + 1) * SA], nA, R1, T1)
                gB = pg.tile([P, cB, R1], BF16, tag="gatB")
                dma_gather_raw(nc.gpsimd, gB[:, :, :],
                               t1_full[HALF:N, 0:R1],
                               idxB_sb[:, g * SB:(g + 1) * SB], nB, R1, T1)
                aldb = aldbp[:, (g - b0) * KB:(g - b0 + 1) * KB, :]

                # ex = exp(leaky_relu(als[s] + ald[d]))
                ep = pe.tile([P, KB, Q, H1], F32, tag="ep")
                nc.vector.tensor_tensor(
                    out=ep[:, 0:bAt],
                    in0=gA[:, :, D1:D1 + H1].rearrange(
                        "p (b q) h -> p b q h", q=Q),
                    in1=aldb[:, 0:bAt, None, :].to_broadcast([P, bAt, Q, H1]),
                    op=OP.add)
                nc.vector.tensor_tensor(
                    out=ep[:, bAt:KB],
                    in0=gB[:, :, D1:D1 + H1].rearrange(
                        "p (b q) h -> p b q h", q=Q),
                    in1=aldb[:, bAt:KB, None, :].to_broadcast([P, bBt, Q, H1]),
                    op=OP.add)
                lr = pe.tile([P, KB, Q, H1], F32, tag="lr")
                nc.scalar.activation(lr[:], ep[:], AF.Copy, scale=NEG_SLOPE)
                nc.vector.tensor_tensor(out=lr[:], in0=lr[:], in1=ep[:],
                                        op=OP.max)
                nc.vector.tensor_tensor(
                    out=lr[:], in0=lr[:],
                    in1=emask_sb[:, g * COLS:(g + 1) * COLS].rearrange(
                        "p (b q) -> p b q", q=Q)[:, :, :, None].to_broadcast(
                        [P, KB, Q, H1]),
                    op=OP.add)
                msg = pm.tile([P, COLS, D1 + H1], BF16, tag="msg")
                nc.scalar.activation(
                    msg[:, :, D1:D1 + H1].rearrange("p (b q) h -> p b q h", q=Q),
                    lr[:], AF.Exp)
                # msg features: ex broadcast over c with unit-stride inner h
                nc.vector.tensor_tensor(
                    out=msg[:, 0:cA, 0:D1].rearrange(
                        "p k (c h) -> p k c h", h=H1),
                    in0=gA[:, :, 0:D1].rearrange("p k (c h) -> p k c h", h=H1),
                    in1=msg[:, 0:cA, D1:D1 + H1][:, :, None, :].to_broadcast(
                        [P, cA, C1, H1]),
                    op=OP.mult)
                nc.vector.tensor_tensor(
                    out=msg[:, cA:COLS, 0:D1].rearrange(
                        "p k (c h) -> p k c h", h=H1),
                    in0=gB[:, :, 0:D1].rearrange("p k (c h) -> p k c h", h=H1),
                    in1=msg[:, cA:COLS, D1:D1 + H1][:, :, None, :].to_broadcast(
                        [P, cB, C1, H1]),
                    op=OP.mult)

                st = pm.tile([P, KB, P], BF16, tag="st")
                nc.sync.dma_start(out=st[:],
                                  in_=stsel[:, g * KB * P:(g + 1) * KB * P])

                ps_g = pc_ps.tile([P, D1 + H1], F32, tag="ps_g")
                for col in range(COLS):
                    nc.tensor.matmul(out=ps_g[:], lhsT=st[:, col // Q, :],
                                     rhs=msg[:, col, :],
                                     start=(col == 0), stop=(col == COLS - 1))

                # epilogue: alpha-normalize, +b1, ELU -> h1 (bf16).
                # PSUM leaves via ACT only (DVE PSUM reads stall under
                # concurrent PE writes).
                hps = pe.tile([P, D1 + H1], F32, tag="hps")
                nc.scalar.activation(hps[:wn], ps_g[:wn], AF.Copy)
                rec = pe.tile([P, H1], F32, tag="rec")
                nc.vector.reciprocal(rec[:wn], hps[:wn, D1:D1 + H1])
                h1f = pg.tile([P, D1], F32, tag="h1f")
                nc.vector.tensor_tensor(
                    out=h1f[:wn].rearrange("p (c h) -> p c h", h=H1),
                    in0=hps[:wn, 0:D1].rearrange("p (c h) -> p c h", h=H1),
                    in1=rec[:wn][:, None, :].to_broadcast([wn, C1, H1]),
                    op=OP.mult)
                nc.vector.tensor_tensor(out=h1f[:wn], in0=h1f[:wn], in1=b1_r[:wn],
                                        op=OP.add)
                # ELU: relu(x) + exp(-relu(-x)) - 1, relus+exp on ACT
                mn = pe.tile([P, D1], F32, tag="mn")
                nc.scalar.activation(mn[:wn], h1f[:wn], AF.Relu, scale=-1.0)
                em = pe.tile([P, D1], F32, tag="em")
                nc.scalar.activation(em[:wn], mn[:wn], AF.Exp, scale=-1.0)
                rl = pe.tile([P, D1], F32, tag="rl")
                nc.scalar.activation(rl[:wn], h1f[:wn], AF.Relu)
                nc.vector.tensor_scalar_add(em[:wn], em[:wn], -1.0)
                h1b = pg.tile([P, D1], BF16, tag="h1b")
                nc.vector.tensor_tensor(out=h1b[:wn], in0=rl[:wn], in1=em[:wn],
                                        op=OP.add)

                # fused layer-2 node-table build
                row2 = pe.tile([P, R2 + H2], BF16, tag="row2")
                ps_x2 = pt_ps.tile([P, D2], F32, tag="ps_x2")
                for c in range(KD):
                    pt = pt_ps.tile([P, P], BF16, tag="pt")
                    nc.tensor.transpose(pt[:], h1b[:, c * P:(c + 1) * P], ident[:])
                    cpt = pe.tile([P, P], BF16, tag="cpt")
                    nc.vector.tensor_copy(cpt[:], pt[:])
                    nc.tensor.matmul(out=ps_x2[:], lhsT=cpt[:], rhs=w2sb[:, c, :],
                                     start=(c == 0), stop=(c == KD - 1))
                x2f = pe.tile([P, D2], F32, tag="x2f")
                nc.scalar.activation(x2f[:wn], ps_x2[:wn], AF.Copy)
                t2m = pe.tile([P, D2], F32, tag="t2m")
                nc.vector.tensor_tensor(out=t2m[:wn], in0=x2f[:wn],
                                        in1=a2s_r[:wn], op=OP.mult)
                with nc.allow_low_precision(reason="16-wide sum to bf16"):
                    nc.vector.tensor_reduce(out=row2[:wn, D2:D2 + H2],
                                            in_=t2m[:wn],
                                            axis=mybir.AxisListType.X,
                                            op=OP.add)
                nc.vector.tensor_tensor(out=t2m[:wn], in0=x2f[:wn],
                                        in1=a2d_r[:wn], op=OP.mult)
                with nc.allow_low_precision(reason="16-wide sum to bf16"):
                    nc.vector.tensor_reduce(out=row2[:wn, D2 + H2:D2 + 2 * H2],
                                            in_=t2m[:wn],
                                            axis=mybir.AxisListType.X,
                                            op=OP.add)
                nc.vector.tensor_copy(row2[:wn, 0:D2], x2f[:wn])
                # row2 = [xp2 | als2 | ald2]; ald2 (col 17) stays local-only
                nc.sync.dma_start(out=t2_loc[w0:w0 + wn, 0:R2 + H2],
                                  in_=row2[:wn, :])
            cctx.close()

          if "h" in stages:
            # ---- AllGather layer-2 table -----------------------------------
            nc.gpsimd.collective_compute(
                "AllGather", OP.bypass, replica_groups=rg,
                ins=[t2_loc.ap()], outs=[t2_full.ap()])

          if "f" in stages:
            # ---- stage F: layer-2 edge phase + log_softmax ------------------
            fctx = ExitStack()
            pal2 = fctx.enter_context(tc.tile_pool(name="pal2", bufs=1))
            paf = fctx.enter_context(tc.tile_pool(name="paf", bufs=6))
            pf = fctx.enter_context(tc.tile_pool(name="pf", bufs=4))
            pfg = fctx.enter_context(tc.tile_pool(name="pfg", bufs=6))
            pf_ps = fctx.enter_context(tc.tile_pool(name="pf_ps", bufs=2,
                                                    space="PSUM"))
            cA = bAt * Q
            cB = bBt * Q
            x2_all = pal2.tile([P, G, D2], F32, tag="x2_all")
            nc.vector.memset(x2_all[:], 0.0)   # last group's tail rows
            for b0 in range(0, G, 2):
              gz = min(2, G - b0)
              ald2p = paf.tile([P, 2 * KB, H2], BF16, tag="ald2p")
              dma_gather_raw(nc.gpsimd, ald2p[:, 0:gz * KB, :],
                             t2_loc[:, R2:R2 + H2],
                             idxN_sb[:, b0 * SBN:(b0 + gz) * SBN],
                             gz * nN, H2, T2)
              for g in range(b0, b0 + gz):
                w0 = g * P
                wn = min(P, B - w0)
                g2A = pfg.tile([P, cA, R2], BF16, tag="gat2A")
                dma_gather_raw(nc.gpsimd, g2A[:, :, :],
                               t2_full[0:HALF, 0:R2],
                               idxA_sb[:, g * SA:(g + 1) * SA], nA, R2, T2)
                g2B = pfg.tile([P, cB, R2], BF16, tag="gat2B")
                dma_gather_raw(nc.gpsimd, g2B[:, :, :],
                               t2_full[HALF:N, 0:R2],
                               idxB_sb[:, g * SB:(g + 1) * SB], nB, R2, T2)
                ald2b = ald2p[:, (g - b0) * KB:(g - b0 + 1) * KB, :]

                ep2 = pf.tile([P, KB, Q, H2], F32, tag="ep2")
                nc.vector.tensor_tensor(
                    out=ep2[:, 0:bAt],
                    in0=g2A[:, :, D2:D2 + H2].rearrange(
                        "p (b q) h -> p b q h", q=Q),
                    in1=ald2b[:, 0:bAt, None, :].to_broadcast([P, bAt, Q, H2]),
                    op=OP.add)
                nc.vector.tensor_tensor(
                    out=ep2[:, bAt:KB],
                    in0=g2B[:, :, D2:D2 + H2].rearrange(
                        "p (b q) h -> p b q h", q=Q),
                    in1=ald2b[:, bAt:KB, None, :].to_broadcast([P, bBt, Q, H2]),
                    op=OP.add)
                lr2 = pf.tile([P, KB, Q, H2], F32, tag="lr2")
                nc.scalar.activation(lr2[:], ep2[:], AF.Copy, scale=NEG_SLOPE)
                nc.vector.tensor_tensor(out=lr2[:], in0=lr2[:], in1=ep2[:],
                                        op=OP.max)
                nc.vector.tensor_tensor(
                    out=lr2[:], in0=lr2[:],
                    in1=emask_sb[:, g * COLS:(g + 1) * COLS].rearrange(
                        "p (b q) -> p b q", q=Q)[:, :, :, None].to_broadcast(
                        [P, KB, Q, H2]),
                    op=OP.add)
                msg2 = pf.tile([P, COLS, R2], BF16, tag="msg2")
                nc.scalar.activation(
                    msg2[:, :, D2:D2 + H2].rearrange("p (b q) h -> p b q h", q=Q),
                    lr2[:], AF.Exp)
                nc.vector.tensor_tensor(
                    out=msg2[:, 0:cA, 0:D2],
                    in0=g2A[:, :, 0:D2],
                    in1=msg2[:, 0:cA, D2:D2 + H2].to_broadcast([P, cA, D2]),
                    op=OP.mult)
                nc.vector.tensor_tensor(
                    out=msg2[:, cA:COLS, 0:D2],
                    in0=g2B[:, :, 0:D2],
                    in1=msg2[:, cA:COLS, D2:D2 + H2].to_broadcast([P, cB, D2]),
                    op=OP.mult)

                st2 = pf.tile([P, KB, P], BF16, tag="st2")
                nc.sync.dma_start(out=st2[:],
                                  in_=stsel[:, g * KB * P:(g + 1) * KB * P])

                ps2 = pf_ps.tile([P, R2], F32, tag="ps2")
                for col in range(COLS):
                    nc.tensor.matmul(out=ps2[:], lhsT=st2[:, col // Q, :],
                                     rhs=msg2[:, col, :],
                                     start=(col == 0), stop=(col == COLS - 1))

                rec2 = pf.tile([P, H2], F32, tag="rec2")
                nc.vector.reciprocal(rec2[:wn], ps2[:wn, D2:D2 + H2])
                x2 = pf.tile([P, D2], F32, tag="x2")
                nc.scalar.activation(x2[:wn], ps2[:wn, 0:D2], AF.Copy,
                                     scale=rec2[:wn])
                nc.vector.tensor_tensor(out=x2_all[:wn, g, :], in0=x2[:wn],
                                        in1=b2_r[:wn], op=OP.add)

            # deferred log_softmax over all groups (one Exp + one Ln pass)
            mx = pal2.tile([P, G, 1], F32, tag="mx")
            nc.vector.tensor_reduce(out=mx[:], in_=x2_all[:],
                                    axis=mybir.AxisListType.X, op=OP.max)
            xs = pal2.tile([P, G, D2], F32, tag="xs")
            nc.vector.tensor_tensor(out=xs[:], in0=x2_all[:],
                                    in1=mx[:].to_broadcast([P, G, D2]),
                                    op=OP.subtract)
            es = pal2.tile([P, G, D2], F32, tag="es")
            nc.scalar.activation(es[:], xs[:], AF.Exp)
            ssum = pal2.tile([P, G, 1], F32, tag="ssum")
            nc.vector.tensor_reduce(out=ssum[:], in_=es[:],
                                    axis=mybir.AxisListType.X, op=OP.add)
            ls = pal2.tile([P, G, 1], F32, tag="ls")
            nc.scalar.activation(ls[:], ssum[:], AF.Ln)
            ot = pal2.tile([P, G, D2], F32, tag="ot")
            nc.vector.tensor_tensor(out=ot[:], in0=xs[:],
                                    in1=ls[:].to_broadcast([P, G, D2]),
                                    op=OP.subtract)
            GF = B // P                       # full 128-row groups
            nc.sync.dma_start(
                out=out2[0:GF * P, :].rearrange("(g p) f -> p g f", p=P),
                in_=ot[:, 0:GF, :])
            if B % P:
                nc.sync.dma_start(out=out2[GF * P:B, :],
                                  in_=ot[:B - GF * P, GF, :])
            fctx.close()

    # Align gather queue_num with tile's DMASW lane rotation (final
    # post-scheduling order, lane = idx%8, 8 lanes): queue = idx%4 keeps
    # each lane on exactly one queue while 4 transfers overlap.
    from concourse.tile_scheduler import DMAInst
    qi = 0

    def _fix_queues(bb):
        nonlocal qi
        for inst in bb.instructions:
            if (isinstance(inst, DMAInst)
                    and inst.engine == mybir.EngineType.Pool):
                assert isinstance(inst, mybir.InstDMAGatherAnt), inst
                inst.queue_num = qi % 4
                qi += 1
            for attr in ("body_bb", "then_bb", "else_bb"):
                sub = getattr(inst, attr, None)
                if sub is not None:
                    _fix_queues(sub)

    for bb in nc.m.functions[0].blocks:
        _fix_queues(bb)

    nc.compile()
    return nc


def make_in_maps(dims: Dims, inputs: dict, per_core_meta):
    """Per-core input maps. W1/a1*/b1 columns are reordered to the
    (c,h)-interleaved layout the kernel uses internally (pure relayout)."""
    H1, C1, D1 = dims.H1, dims.C1, dims.D1
    perm = np.arange(D1).reshape(H1, C1).T.reshape(-1)   # [h*C+c] -> [c*H+h]
    x = np.asarray(inputs["x"], dtype=np.float32)
    W2 = np.asarray(inputs["W2"], np.float32)
    reps = {
        "W1": np.ascontiguousarray(
            np.asarray(inputs["W1"], np.float32)[:, perm]).astype(
                ml_dtypes.bfloat16),
        "a1s": np.ascontiguousarray(
            np.asarray(inputs["a1_src"], np.float32).reshape(-1)[perm]),
        "a1d": np.ascontiguousarray(
            np.asarray(inputs["a1_dst"], np.float32).reshape(-1)[perm]),
        "b1": np.ascontiguousarray(
            np.asarray(inputs["b1"], np.float32).reshape(-1)[perm]),
        "W2": np.ascontiguousarray(W2[perm, :]).astype(ml_dtypes.bfloat16),
        "a2s": np.asarray(inputs["a2_src"], np.float32).reshape(-1),
        "a2d": np.asarray(inputs["a2_dst"], np.float32).reshape(-1),
        "b2": np.asarray(inputs["b2"], np.float32).reshape(-1),
    }
    in_maps = []
    B = dims.B
    for k in range(dims.NC):
        m = dict(reps)
        m["xT"] = np.ascontiguousarray(
            x[k * B:(k + 1) * B, :].T).astype(ml_dtypes.bfloat16)
        m.update(per_core_meta[k])
        in_maps.append(m)
    return in_maps


_CACHE = {}


def _get_program(dims: Dims):
    key = (dims.N, dims.E, dims.NC, dims.bA, dims.bB)
    if key not in _CACHE:
        _CACHE[key] = build_program(dims)
    return _CACHE[key]


def kernel(x: np.ndarray, edge_index: np.ndarray, W1, a1_src, a1_dst, b1,
           W2, a2_src, a2_dst, b2) -> np.ndarray:
    x = np.asarray(x)
    edge_index = np.asarray(edge_index)
    dims = Dims(N=x.shape[0], E=edge_index.shape[1], n_cores=8)
    per_core = host_prep(dims, edge_index)
    nc = _get_program(dims)
    in_maps = make_in_maps(
        dims,
        dict(x=x, edge_index=edge_index, W1=W1, a1_src=a1_src, a1_dst=a1_dst,
             b1=b1, W2=W2, a2_src=a2_src, a2_dst=a2_dst, b2=b2),
        per_core)
    res = run_bass_kernel_spmd(nc, in_maps, core_ids=list(range(dims.NC)))
    out = np.concatenate([r["out2"] for r in res.results], axis=0)
    return out.astype(np.float32)



# revision 30
# speedup vs baseline: 1.0746x; 1.0746x over previous
"""2-layer GAT (PyG GATConv style) distributed across 8 TRN2 NeuronCores.

Sharding (per hint): nodes partitioned into 8 contiguous destination blocks.
Per core:
  A. xp1 = x_blk @ W1 and per-node attention logits for its own block are
     packed into a per-node gather-row table (bf16, 768B pitch,
     row = [xp1 (256, (c,h)-interleaved) | als1 (4)]); dst-side logits go to
     a 256B-pitch side table. The row table is AllGathered so every core
     holds all N rows.
  C. layer-1 edge phase over the core's own destination block. Edges are
     bucketed by 128-wide destination windows ("groups") and, within a
     group, into bins of Q=4 slots sharing one destination, laid out on
     (partition, bin-tile) cells -- so dst-side data broadcasts along the
     free dim with unit-stride APs. Source rows are fetched with per-group
     dma_gather calls (int16 indices, table split in two <32768-row
     halves; each call stays under the 128-entry SWDGE FIFO and calls
     rotate across the 4 SWDGE queues so transfers overlap); per-edge
     softmax numerators ex = exp(leaky_relu(als[s]+ald[d])) are computed
     on DVE/ACT (leaky-relu scale and all PSUM reads ride the Scalar
     engine), and messages are segment-summed by host-precomputed 0/1
     selector matmuls on the TensorEngine (PSUM accumulation per group).
     The softmax normalizer rides as extra rhs columns of the same
     matmul, so alpha normalization is a per-node epilogue. ELU and the
     layer-2 node table are fused into the epilogue; that table is
     AllGathered.
  F. layer-2 edge phase (same structure, 17-wide rows); log_softmax is a
     single deferred pass over all groups at the end.

Host preprocessing is index-only (self-loops, bucketing, padding,
selector one-hots, relayout); all float math runs on device.
"""

import math
from contextlib import ExitStack

import numpy as np
import ml_dtypes

import concourse.bass as bass
import concourse.tile as tile
from concourse import bacc, mybir
from concourse.bass_utils import run_bass_kernel_spmd
from concourse.masks import make_identity

F32 = mybir.dt.float32
BF16 = mybir.dt.bfloat16
F8 = mybir.dt.float8e4
I16 = mybir.dt.int16
AF = mybir.ActivationFunctionType
OP = mybir.AluOpType

P = 128
Q = 4                 # slots per bin (same-dst edges per partition-cell)
NEG_SLOPE = 0.2


_GQ = {"i": 0}


def _next_q(nq=4):
    q = _GQ["i"] % nq
    _GQ["i"] += 1
    return q


def dma_gather_raw(gp, out_ap, in_ap, idxs_ap, num_idxs, elem_size,
                   elem_step, single_packet=None, queue_num=None):
    """BassGpSimd.dma_gather minus the payload%256 assert (pitch must still
    be a 256B multiple; verified on HW that arbitrary payload works).

    num_idxs must stay <= ~2032: the per-queue SWDGE FIFO holds 128
    entries and a call consumes num_idxs/16+1. Rotating queue_num lets
    transfers from consecutive calls overlap (one outstanding per queue).
    """
    from concourse._compat import exact_div
    if single_packet is None:
        # single-packet mode breaks somewhere between 1024 and 1536 indices
        single_packet = num_idxs <= 1024
    if queue_num is None:
        queue_num = _next_q()
    assert num_idxs <= 2032, num_idxs
    assert idxs_ap.dtype == mybir.dt.int16
    assert in_ap.dtype == out_ap.dtype
    stride_bytes = elem_step * mybir.dt.size(in_ap.dtype)
    stride_bytes_256 = exact_div(stride_bytes, 256)
    assert stride_bytes_256 < 256
    _in_ap = gp.lower_ap_dma(in_ap, for_custom_bir_dma=True)
    _idxs_ap = gp.lower_ap(idxs_ap)
    _out_ap = gp.lower_ap(out_ap)
    return gp.add_instruction(
        mybir.InstDMAGatherAnt(
            name=gp.bass.get_next_instruction_name(),
            ins=[*_in_ap, _idxs_ap, gp.lower_val_access(gp.to_reg(num_idxs))],
            outs=[_out_ap],
            transpose=False,
            num_idxs=num_idxs,
            elem_size=elem_size,
            stride_bytes_256=stride_bytes_256,
            gen_mode=0,
            single_packet=single_packet,
            queue_num=queue_num,
            sbuf_tokens_per_rank=0,
            sbuf_free_dim_per_rank=0,
            sbuf_free_dim_pad_per_rank=0,
            sbuf_byte_offset=0,
        ))


class Dims:
    def __init__(self, N, E, n_cores, H1=4, C1=64, H2=1, C2=16, F_in=256):
        self.N, self.E, self.NC = N, E, n_cores
        self.F_in = F_in
        self.H1, self.C1, self.H2, self.C2 = H1, C1, H2, C2
        self.D1 = H1 * C1
        self.D2 = H2 * C2
        self.B = N // n_cores
        self.G = math.ceil(self.B / P)
        self.HALF = min(25000, (N + 1) // 2)  # int16 table split point
        self.R1 = self.D1 + self.H1          # gathered row 1: xp1|als1
        self.T1 = 384                         # table-1 pitch (768B bf16)
        self.R2 = self.D2 + self.H2          # gathered row 2: xp2|als2
        self.T2 = 128                         # table-2 pitch (256B bf16)
        self.TA = 128                         # ald side-table pitch (256B)
        self.bA = None   # bin-tiles for src-half A (set by host_prep)
        self.bB = None
        self.KB = None   # bA + bB
        self.COLS = None  # KB * Q edge columns per group


def _wrap_idx16(flat):
    """index list -> [128, ceil(n/16)] int16 SBUF image (16-partition wrap,
    replicated for the 8 Q7 cores)."""
    n = len(flat)
    S = math.ceil(n / 16)
    a = np.zeros((16, S), np.int16)
    i = np.arange(n)
    a[i % 16, i // 16] = flat
    return np.tile(a, (8, 1))


def host_prep(dims: Dims, edge_index: np.ndarray):
    """Index-only preprocessing: self-loops, per-core dst blocks, 128-dst
    windows, same-dst bins of Q slots split by src half, padding, int16
    index images."""
    N, NC, B, G = dims.N, dims.NC, dims.B, dims.G
    HALF = dims.HALF
    loops = np.arange(N, dtype=np.int64)
    src = np.concatenate([edge_index[0].astype(np.int64), loops])
    dst = np.concatenate([edge_index[1].astype(np.int64), loops])

    # per (core, group, half): list of (dst_local, [srcs]) bins
    cores = []
    bmaxA = bmaxB = 1
    for k in range(NC):
        lo, hi = k * B, (k + 1) * B
        m = (dst >= lo) & (dst < hi)
        s_k = src[m]
        d_k = dst[m] - lo
        order = np.lexsort((s_k >= HALF, d_k))
        s_k, d_k = s_k[order], d_k[order]
        half_k = (s_k >= HALF).astype(np.int64)
        groups = []
        for g in range(G):
            gmask = (d_k // P) == g
            sg, dg, hg = s_k[gmask], d_k[gmask] - g * P, half_k[gmask]
            binsA, binsB = [], []
            for h, bins in ((0, binsA), (1, binsB)):
                hm = hg == h
                sh, dh = sg[hm], dg[hm]
                # consecutive same-dst runs -> bins of <= Q edges
                ptr = 0
                n = len(sh)
                while ptr < n:
                    d0 = dh[ptr]
                    end = ptr
                    while end < n and dh[end] == d0 and end - ptr < Q:
                        end += 1
                    bins.append((int(d0), sh[ptr:end]))
                    ptr = end
            groups.append((binsA, binsB))
            bmaxA = max(bmaxA, math.ceil(len(binsA) / P))
            bmaxB = max(bmaxB, math.ceil(len(binsB) / P))
        cores.append(groups)
    dims.bA, dims.bB = bmaxA, bmaxB
    dims.KB = bmaxA + bmaxB
    dims.COLS = dims.KB * Q

    G_, KB, COLS = G, dims.KB, dims.COLS
    per_core = []
    for k in range(NC):
        # slot-level arrays
        srcl = np.zeros((P, G_, COLS), np.int64)   # half-local src id
        emask = np.full((P, G_, COLS), -150.0, np.float32)  # pad-slot mask
        dstl = np.full((P, G_, KB), -1.0, np.float32)  # per-bin local dst
        binid = np.zeros((P, G_, KB), np.int64)    # per-bin LOCAL dst row id
        for g in range(G_):
            binsA, binsB = cores[k][g]
            for hoff, bins in ((0, binsA), (dims.bA, binsB)):
                for b, (d0, ss) in enumerate(bins):
                    p = b % P
                    bt = hoff + b // P
                    dstl[p, g, bt] = float(d0)
                    binid[p, g, bt] = g * P + d0
                    for q, s in enumerate(ss):
                        srcl[p, g, bt * Q + q] = s if s < HALF else s - HALF
                        emask[p, g, bt * Q + q] = 0.0
        # int16 gather-index images per group, concatenated along free dim
        SA = dims.bA * Q * P // 16            # idxA image cols per group
        SB = dims.bB * Q * P // 16
        SBN = KB * P // 16
        idxA = np.zeros((P, G_ * SA), np.int16)
        idxB = np.zeros((P, G_ * SB), np.int16)
        idxN = np.zeros((P, G_ * SBN), np.int16)
        for g in range(G_):
            cA = dims.bA * Q
            flatA = srcl[:, g, 0:cA].T.reshape(-1)      # i = c*128 + p
            flatB = srcl[:, g, cA:COLS].T.reshape(-1)
            flatN = binid[:, g, :].T.reshape(-1)
            idxA[:, g * SA:(g + 1) * SA] = _wrap_idx16(flatA)
            idxB[:, g * SB:(g + 1) * SB] = _wrap_idx16(flatB)
            idxN[:, g * SBN:(g + 1) * SBN] = _wrap_idx16(flatN)
        # selector one-hots: stsel[p, (g,bt,d)] = 1 iff dstl[p,g,bt] == d
        stsel = np.zeros((P, G_, KB, P), ml_dtypes.bfloat16)
        pp, gg, bb = np.nonzero(dstl >= 0)
        stsel[pp, gg, bb, dstl[pp, gg, bb].astype(np.int64)] = 1.0
        per_core.append(dict(
            idxA=idxA, idxB=idxB, idxN=idxN,
            stsel=np.ascontiguousarray(stsel.reshape(P, G_ * KB * P)),
            emask=np.ascontiguousarray(
                emask.reshape(P, G_ * COLS)).astype(ml_dtypes.bfloat16),
        ))
    return per_core


def build_program(dims: Dims, replicas: int = 1, stages: str = "agchf"):
    N, NC, B, G = dims.N, dims.NC, dims.B, dims.G
    F_in, D1, D2, H1, H2 = dims.F_in, dims.D1, dims.D2, dims.H1, dims.H2
    C1 = dims.C1
    R1, T1, R2, T2, TA = dims.R1, dims.T1, dims.R2, dims.T2, dims.TA
    bAt, bBt, KB, COLS = dims.bA, dims.bB, dims.KB, dims.COLS
    KF = F_in // P
    KD = D1 // P
    SA = bAt * Q * P // 16
    SB = bBt * Q * P // 16
    SBN = KB * P // 16
    nA = bAt * Q * P     # indices per gather call A
    nB = bBt * Q * P
    nN = KB * P
    HALF = dims.HALF

    _GQ["i"] = 0
    nc = bacc.Bacc("TRN2", target_bir_lowering=False, debug=False,
                   enable_asserts=False, num_devices=NC,
                   num_swdge_queues=4)

    xT = nc.dram_tensor("xT", [F_in, B], BF16, kind="ExternalInput")
    W1 = nc.dram_tensor("W1", [F_in, D1], BF16, kind="ExternalInput")
    a1s = nc.dram_tensor("a1s", [D1], F32, kind="ExternalInput")
    a1d = nc.dram_tensor("a1d", [D1], F32, kind="ExternalInput")
    b1 = nc.dram_tensor("b1", [D1], F32, kind="ExternalInput")
    W2 = nc.dram_tensor("W2", [D1, D2], BF16, kind="ExternalInput")
    a2s = nc.dram_tensor("a2s", [D2], F32, kind="ExternalInput")
    a2d = nc.dram_tensor("a2d", [D2], F32, kind="ExternalInput")
    b2 = nc.dram_tensor("b2", [D2], F32, kind="ExternalInput")
    idxA = nc.dram_tensor("idxA", [P, G * SA], I16, kind="ExternalInput")
    idxB = nc.dram_tensor("idxB", [P, G * SB], I16, kind="ExternalInput")
    idxN = nc.dram_tensor("idxN", [P, G * SBN], I16, kind="ExternalInput")
    stsel = nc.dram_tensor("stsel", [P, G * KB * P], BF16,
                           kind="ExternalInput")
    emask = nc.dram_tensor("emask", [P, G * COLS], BF16, kind="ExternalInput")
    out2 = nc.dram_tensor("out2", [B, D2], F32, kind="ExternalOutput")

    t1_loc = nc.dram_tensor("t1_loc", [B, T1], BF16)
    t1_full = nc.dram_tensor("t1_full", [N, T1], BF16, addr_space="Shared")
    ald1t = nc.dram_tensor("ald1t", [B, TA], BF16)
    t2_loc = nc.dram_tensor("t2_loc", [B, T2], BF16)
    t2_full = nc.dram_tensor("t2_full", [N, T2], BF16, addr_space="Shared")

    rg = [list(range(NC))]

    with tile.TileContext(nc) as tc, ExitStack() as ctx:
        const = ctx.enter_context(tc.tile_pool(name="const", bufs=1))
        ictx = ExitStack()
        cpsum = ictx.enter_context(tc.tile_pool(name="cpsum", bufs=1,
                                                space="PSUM"))

        ident = const.tile([P, P], BF16, tag="ident")
        make_identity(nc, ident[:])

        w1sb = const.tile([P, KF, D1], BF16, tag="w1sb")
        for c in range(KF):
            nc.sync.dma_start(out=w1sb[:, c, :], in_=W1[c * P:(c + 1) * P, :])
        w2sb = const.tile([P, KD, D2], BF16, tag="w2sb")
        for c in range(KD):
            nc.sync.dma_start(out=w2sb[:, c, :], in_=W2[c * P:(c + 1) * P, :])

        ones_row = const.tile([1, P], F32, tag="ones_row")
        nc.vector.memset(ones_row[:], 1.0)

        def replicate(vec_ap, X, tag):
            vrow = const.tile([1, X], F32, tag=tag + "_row")
            nc.sync.dma_start(out=vrow[:], in_=vec_ap[None, :])
            pr = cpsum.tile([P, X], F32, tag="reppsum")
            nc.tensor.matmul(out=pr[:], lhsT=ones_row[:], rhs=vrow[:],
                             start=True, stop=True)
            rep = const.tile([P, X], F32, tag=tag)
            nc.vector.tensor_copy(rep[:], pr[:])
            return rep

        a1s_r = replicate(a1s, D1, "a1s_r")
        a1d_r = replicate(a1d, D1, "a1d_r")
        b1_r = replicate(b1, D1, "b1_r")
        a2s_r = replicate(a2s, D2, "a2s_r")
        a2d_r = replicate(a2d, D2, "a2d_r")
        b2_r = replicate(b2, D2, "b2_r")

        idxA_sb = const.tile([P, G * SA], I16, tag="idxA_sb")
        nc.sync.dma_start(out=idxA_sb[:], in_=idxA[:, :])
        idxB_sb = const.tile([P, G * SB], I16, tag="idxB_sb")
        nc.sync.dma_start(out=idxB_sb[:], in_=idxB[:, :])
        idxN_sb = const.tile([P, G * SBN], I16, tag="idxN_sb")
        nc.sync.dma_start(out=idxN_sb[:], in_=idxN[:, :])
        emask_sb = const.tile([P, G * COLS], BF16, tag="emask_sb")
        nc.sync.dma_start(out=emask_sb[:], in_=emask[:, :])

        ictx.close()

        for _rep in range(replicas):
          if "a" in stages:
            # ---- stage A: layer-1 node table for own block -----------------
            actx = ExitStack()
            pa = actx.enter_context(tc.tile_pool(name="pa", bufs=3))
            prowa = actx.enter_context(tc.tile_pool(name="prowa", bufs=10))
            pa_ps = actx.enter_context(tc.tile_pool(name="pa_ps", bufs=2,
                                                    space="PSUM"))
            for t in range(G):
                n0 = t * P
                nn = min(P, B - n0)
                xta = pa.tile([P, KF, P], BF16, tag="xta")
                for c in range(KF):
                    nc.sync.dma_start(out=xta[:, c, :nn],
                                      in_=xT[c * P:(c + 1) * P, n0:n0 + nn])
                ps_xp = pa_ps.tile([P, D1], F32, tag="ps_xp")
                for c in range(KF):
                    nc.tensor.matmul(out=ps_xp[:nn, :], lhsT=xta[:, c, :nn],
                                     rhs=w1sb[:, c, :],
                                     start=(c == 0), stop=(c == KF - 1))
                tmp = pa.tile([P, D1], F32, tag="tmpa")
                alf = pa.tile([P, 2 * H1], F32, tag="alf")
                row = prowa.tile([P, R1], BF16, tag="row1")
                rowd = prowa.tile([P, H1], BF16, tag="rowd")
                nc.vector.tensor_tensor(out=tmp[:nn], in0=ps_xp[:nn],
                                        in1=a1s_r[:nn], op=OP.mult)
                # features are (c,h)-interleaved: head h at stride-4 positions
                nc.vector.tensor_reduce(
                    out=alf[:nn, 0:H1],
                    in_=tmp[:nn].rearrange("p (c h) -> p h c", h=H1),
                    axis=mybir.AxisListType.X, op=OP.add)
                nc.vector.tensor_tensor(out=tmp[:nn], in0=ps_xp[:nn],
                                        in1=a1d_r[:nn], op=OP.mult)
                nc.vector.tensor_reduce(
                    out=alf[:nn, H1:2 * H1],
                    in_=tmp[:nn].rearrange("p (c h) -> p h c", h=H1),
                    axis=mybir.AxisListType.X, op=OP.add)
                nc.vector.tensor_copy(row[:nn, D1:D1 + H1], alf[:nn, 0:H1])
                nc.vector.tensor_copy(row[:nn, 0:D1], ps_xp[:nn])
                nc.vector.tensor_copy(rowd[:nn, :], alf[:nn, H1:2 * H1])
                nc.sync.dma_start(out=t1_loc[n0:n0 + nn, 0:R1], in_=row[:nn, :])
                nc.sync.dma_start(out=ald1t[n0:n0 + nn, 0:H1], in_=rowd[:nn, :])
            actx.close()

          if "g" in stages:
            # ---- AllGather layer-1 table -----------------------------------
            nc.gpsimd.collective_compute(
                "AllGather", OP.bypass, replica_groups=rg,
                ins=[t1_loc.ap()], outs=[t1_full.ap()])

          if "c" in stages:
            # ---- stage C: layer-1 edge phase + fused layer-2 table ---------
            cctx = ExitStack()
            pal = cctx.enter_context(tc.tile_pool(name="pal", bufs=6))
            pg = cctx.enter_context(tc.tile_pool(name="pg", bufs=4))
            pm = cctx.enter_context(tc.tile_pool(name="pm", bufs=3))
            pe = cctx.enter_context(tc.tile_pool(name="pe", bufs=3))
            prow2 = cctx.enter_context(tc.tile_pool(name="prow2", bufs=12))
            pc_ps = cctx.enter_context(tc.tile_pool(name="pc_ps", bufs=3,
                                                    space="PSUM"))
            pt_ps = cctx.enter_context(tc.tile_pool(name="pt_ps", bufs=2,
                                                    space="PSUM"))
            cA = bAt * Q
            cB = bBt * Q
            # per-bin dst logits, two groups per call (<=2032-idx FIFO cap)
            for b0 in range(0, G, 2):
              gz = min(2, G - b0)
              aldbp = pal.tile([P, 2 * KB, H1], BF16, tag="aldbp")
              dma_gather_raw(nc.gpsimd, aldbp[:, 0:gz * KB, :],
                             ald1t[:, 0:H1],
                             idxN_sb[:, b0 * SBN:(b0 + gz) * SBN],
                             gz * nN, H1, TA)
              for g in range(b0, b0 + gz):
                w0 = g * P
                wn = min(P, B - w0)
                gA = pg.tile([P, cA, R1], BF16, tag="gatA")
                dma_gather_raw(nc.gpsimd, gA[:, :, :],
                               t1_full[0:HALF, 0:R1],
                               idxA_sb[:, g * SA:(g + 1) * SA], nA, R1, T1)
                gB = pg.tile([P, cB, R1], BF16, tag="gatB")
                dma_gather_raw(nc.gpsimd, gB[:, :, :],
                               t1_full[HALF:N, 0:R1],
                               idxB_sb[:, g * SB:(g + 1) * SB], nB, R1, T1)
                aldb = aldbp[:, (g - b0) * KB:(g - b0 + 1) * KB, :]

                # ex = exp(leaky_relu(als[s] + ald[d]))
                ep = pe.tile([P, KB, Q, H1], F32, tag="ep")
                nc.vector.tensor_tensor(
                    out=ep[:, 0:bAt],
                    in0=gA[:, :, D1:D1 + H1].rearrange(
                        "p (b q) h -> p b q h", q=Q),
                    in1=aldb[:, 0:bAt, None, :].to_broadcast([P, bAt, Q, H1]),
                    op=OP.add)
                nc.vector.tensor_tensor(
                    out=ep[:, bAt:KB],
                    in0=gB[:, :, D1:D1 + H1].rearrange(
                        "p (b q) h -> p b q h", q=Q),
                    in1=aldb[:, bAt:KB, None, :].to_broadcast([P, bBt, Q, H1]),
                    op=OP.add)
                lr = pe.tile([P, KB, Q, H1], F32, tag="lr")
                nc.scalar.activation(lr[:], ep[:], AF.Copy, scale=NEG_SLOPE)
                nc.vector.tensor_tensor(out=lr[:], in0=lr[:], in1=ep[:],
                                        op=OP.max)
                nc.vector.tensor_tensor(
                    out=lr[:], in0=lr[:],
                    in1=emask_sb[:, g * COLS:(g + 1) * COLS].rearrange(
                        "p (b q) -> p b q", q=Q)[:, :, :, None].to_broadcast(
                        [P, KB, Q, H1]),
                    op=OP.add)
                msg = pm.tile([P, COLS, D1 + H1], BF16, tag="msg")
                nc.scalar.activation(
                    msg[:, :, D1:D1 + H1].rearrange("p (b q) h -> p b q h", q=Q),
                    lr[:], AF.Exp)
                # msg features: ex broadcast over c with unit-stride inner h
                nc.vector.tensor_tensor(
                    out=msg[:, 0:cA, 0:D1].rearrange(
                        "p k (c h) -> p k c h", h=H1),
                    in0=gA[:, :, 0:D1].rearrange("p k (c h) -> p k c h", h=H1),
                    in1=msg[:, 0:cA, D1:D1 + H1][:, :, None, :].to_broadcast(
                        [P, cA, C1, H1]),
                    op=OP.mult)
                nc.vector.tensor_tensor(
                    out=msg[:, cA:COLS, 0:D1].rearrange(
                        "p k (c h) -> p k c h", h=H1),
                    in0=gB[:, :, 0:D1].rearrange("p k (c h) -> p k c h", h=H1),
                    in1=msg[:, cA:COLS, D1:D1 + H1][:, :, None, :].to_broadcast(
                        [P, cB, C1, H1]),
                    op=OP.mult)

                st = pm.tile([P, KB, P], BF16, tag="st")
                nc.sync.dma_start(out=st[:],
                                  in_=stsel[:, g * KB * P:(g + 1) * KB * P])

                ps_g = pc_ps.tile([P, D1 + H1], F32, tag="ps_g")
                for col in range(COLS):
                    nc.tensor.matmul(out=ps_g[:], lhsT=st[:, col // Q, :],
                                     rhs=msg[:, col, :],
                                     start=(col == 0), stop=(col == COLS - 1))

                # epilogue: alpha-normalize, +b1, ELU -> h1 (bf16).
                # PSUM leaves via ACT only (DVE PSUM reads stall under
                # concurrent PE writes).
                hps = pe.tile([P, D1 + H1], F32, tag="hps")
                nc.scalar.activation(hps[:wn], ps_g[:wn], AF.Copy)
                rec = pe.tile([P, H1], F32, tag="rec")
                nc.vector.reciprocal(rec[:wn], hps[:wn, D1:D1 + H1])
                h1f = pg.tile([P, D1], F32, tag="h1f")
                nc.vector.tensor_tensor(
                    out=h1f[:wn].rearrange("p (c h) -> p c h", h=H1),
                    in0=hps[:wn, 0:D1].rearrange("p (c h) -> p c h", h=H1),
                    in1=rec[:wn][:, None, :].to_broadcast([wn, C1, H1]),
                    op=OP.mult)
                nc.vector.tensor_tensor(out=h1f[:wn], in0=h1f[:wn], in1=b1_r[:wn],
                                        op=OP.add)
                # ELU: relu(x) + exp(-relu(-x)) - 1, relus+exp on ACT
                mn = pe.tile([P, D1], F32, tag="mn")
                nc.scalar.activation(mn[:wn], h1f[:wn], AF.Relu, scale=-1.0)
                em = pe.tile([P, D1], F32, tag="em")
                nc.scalar.activation(em[:wn], mn[:wn], AF.Exp, scale=-1.0)
                rl = pe.tile([P, D1], F32, tag="rl")
                nc.scalar.activation(rl[:wn], h1f[:wn], AF.Relu)
                nc.vector.tensor_scalar_add(em[:wn], em[:wn], -1.0)
                h1b = pg.tile([P, D1], BF16, tag="h1b")
                nc.vector.tensor_tensor(out=h1b[:wn], in0=rl[:wn], in1=em[:wn],
                                        op=OP.add)

                # fused layer-2 node-table build
                row2 = prow2.tile([P, R2 + H2], BF16, tag="row2")
                ps_x2 = pt_ps.tile([P, D2], F32, tag="ps_x2")
                for c in range(KD):
                    pt = pt_ps.tile([P, P], BF16, tag="pt")
                    nc.tensor.transpose(pt[:], h1b[:, c * P:(c + 1) * P], ident[:])
                    cpt = pe.tile([P, P], BF16, tag="cpt")
                    nc.vector.tensor_copy(cpt[:], pt[:])
                    nc.tensor.matmul(out=ps_x2[:], lhsT=cpt[:], rhs=w2sb[:, c, :],
                                     start=(c == 0), stop=(c == KD - 1))
                x2f = pe.tile([P, D2], F32, tag="x2f")
                nc.scalar.activation(x2f[:wn], ps_x2[:wn], AF.Copy)
                t2m = pe.tile([P, D2], F32, tag="t2m")
                nc.vector.tensor_tensor(out=t2m[:wn], in0=x2f[:wn],
                                        in1=a2s_r[:wn], op=OP.mult)
                with nc.allow_low_precision(reason="16-wide sum to bf16"):
                    nc.vector.tensor_reduce(out=row2[:wn, D2:D2 + H2],
                                            in_=t2m[:wn],
                                            axis=mybir.AxisListType.X,
                                            op=OP.add)
                nc.vector.tensor_tensor(out=t2m[:wn], in0=x2f[:wn],
                                        in1=a2d_r[:wn], op=OP.mult)
                with nc.allow_low_precision(reason="16-wide sum to bf16"):
                    nc.vector.tensor_reduce(out=row2[:wn, D2 + H2:D2 + 2 * H2],
                                            in_=t2m[:wn],
                                            axis=mybir.AxisListType.X,
                                            op=OP.add)
                nc.vector.tensor_copy(row2[:wn, 0:D2], x2f[:wn])
                # row2 = [xp2 | als2 | ald2]; ald2 (col 17) stays local-only
                nc.sync.dma_start(out=t2_loc[w0:w0 + wn, 0:R2 + H2],
                                  in_=row2[:wn, :])
            cctx.close()

          if "h" in stages:
            # ---- AllGather layer-2 table -----------------------------------
            nc.gpsimd.collective_compute(
                "AllGather", OP.bypass, replica_groups=rg,
                ins=[t2_loc.ap()], outs=[t2_full.ap()])

          if "f" in stages:
            # ---- stage F: layer-2 edge phase + log_softmax ------------------
            fctx = ExitStack()
            pal2 = fctx.enter_context(tc.tile_pool(name="pal2", bufs=1))
            paf = fctx.enter_context(tc.tile_pool(name="paf", bufs=6))
            pf = fctx.enter_context(tc.tile_pool(name="pf", bufs=4))
            pfg = fctx.enter_context(tc.tile_pool(name="pfg", bufs=6))
            pf_ps = fctx.enter_context(tc.tile_pool(name="pf_ps", bufs=4,
                                                    space="PSUM"))
            cA = bAt * Q
            cB = bBt * Q
            x2_all = pal2.tile([P, G, D2], F32, tag="x2_all")
            nc.vector.memset(x2_all[:], 0.0)   # last group's tail rows
            for b0 in range(0, G, 2):
              gz = min(2, G - b0)
              ald2p = paf.tile([P, 2 * KB, H2], BF16, tag="ald2p")
              dma_gather_raw(nc.gpsimd, ald2p[:, 0:gz * KB, :],
                             t2_loc[:, R2:R2 + H2],
                             idxN_sb[:, b0 * SBN:(b0 + gz) * SBN],
                             gz * nN, H2, T2)
              for g in range(b0, b0 + gz):
                w0 = g * P
                wn = min(P, B - w0)
                g2A = pfg.tile([P, cA, R2], BF16, tag="gat2A")
                dma_gather_raw(nc.gpsimd, g2A[:, :, :],
                               t2_full[0:HALF, 0:R2],
                               idxA_sb[:, g * SA:(g + 1) * SA], nA, R2, T2)
                g2B = pfg.tile([P, cB, R2], BF16, tag="gat2B")
                dma_gather_raw(nc.gpsimd, g2B[:, :, :],
                               t2_full[HALF:N, 0:R2],
                               idxB_sb[:, g * SB:(g + 1) * SB], nB, R2, T2)
                ald2b = ald2p[:, (g - b0) * KB:(g - b0 + 1) * KB, :]

                ep2 = pf.tile([P, KB, Q, H2], F32, tag="ep2")
                nc.vector.tensor_tensor(
                    out=ep2[:, 0:bAt],
                    in0=g2A[:, :, D2:D2 + H2].rearrange(
                        "p (b q) h -> p b q h", q=Q),
                    in1=ald2b[:, 0:bAt, None, :].to_broadcast([P, bAt, Q, H2]),
                    op=OP.add)
                nc.vector.tensor_tensor(
                    out=ep2[:, bAt:KB],
                    in0=g2B[:, :, D2:D2 + H2].rearrange(
                        "p (b q) h -> p b q h", q=Q),
                    in1=ald2b[:, bAt:KB, None, :].to_broadcast([P, bBt, Q, H2]),
                    op=OP.add)
                lr2 = pf.tile([P, KB, Q, H2], F32, tag="lr2")
                nc.scalar.activation(lr2[:], ep2[:], AF.Copy, scale=NEG_SLOPE)
                nc.vector.tensor_tensor(out=lr2[:], in0=lr2[:], in1=ep2[:],
                                        op=OP.max)
                nc.vector.tensor_tensor(
                    out=lr2[:], in0=lr2[:],
                    in1=emask_sb[:, g * COLS:(g + 1) * COLS].rearrange(
                        "p (b q) -> p b q", q=Q)[:, :, :, None].to_broadcast(
                        [P, KB, Q, H2]),
                    op=OP.add)
                msg2 = pf.tile([P, COLS, R2], BF16, tag="msg2")
                nc.scalar.activation(
                    msg2[:, :, D2:D2 + H2].rearrange("p (b q) h -> p b q h", q=Q),
                    lr2[:], AF.Exp)
                nc.vector.tensor_tensor(
                    out=msg2[:, 0:cA, 0:D2],
                    in0=g2A[:, :, 0:D2],
                    in1=msg2[:, 0:cA, D2:D2 + H2].to_broadcast([P, cA, D2]),
                    op=OP.mult)
                nc.vector.tensor_tensor(
                    out=msg2[:, cA:COLS, 0:D2],
                    in0=g2B[:, :, 0:D2],
                    in1=msg2[:, cA:COLS, D2:D2 + H2].to_broadcast([P, cB, D2]),
                    op=OP.mult)

                st2 = pf.tile([P, KB, P], BF16, tag="st2")
                nc.sync.dma_start(out=st2[:],
                                  in_=stsel[:, g * KB * P:(g + 1) * KB * P])

                ps2 = pf_ps.tile([P, R2], F32, tag="ps2")
                for col in range(COLS):
                    nc.tensor.matmul(out=ps2[:], lhsT=st2[:, col // Q, :],
                                     rhs=msg2[:, col, :],
                                     start=(col == 0), stop=(col == COLS - 1))

                rec2 = pf.tile([P, H2], F32, tag="rec2")
                nc.vector.reciprocal(rec2[:wn], ps2[:wn, D2:D2 + H2])
                x2 = pf.tile([P, D2], F32, tag="x2")
                nc.scalar.activation(x2[:wn], ps2[:wn, 0:D2], AF.Copy,
                                     scale=rec2[:wn])
                nc.vector.tensor_tensor(out=x2_all[:wn, g, :], in0=x2[:wn],
                                        in1=b2_r[:wn], op=OP.add)

            # deferred log_softmax over all groups (one Exp + one Ln pass)
            mx = pal2.tile([P, G, 1], F32, tag="mx")
            nc.vector.tensor_reduce(out=mx[:], in_=x2_all[:],
                                    axis=mybir.AxisListType.X, op=OP.max)
            xs = pal2.tile([P, G, D2], F32, tag="xs")
            nc.vector.tensor_tensor(out=xs[:], in0=x2_all[:],
                                    in1=mx[:].to_broadcast([P, G, D2]),
                                    op=OP.subtract)
            es = pal2.tile([P, G, D2], F32, tag="es")
            nc.scalar.activation(es[:], xs[:], AF.Exp)
            ssum = pal2.tile([P, G, 1], F32, tag="ssum")
            nc.vector.tensor_reduce(out=ssum[:], in_=es[:],
                                    axis=mybir.AxisListType.X, op=OP.add)
            ls = pal2.tile([P, G, 1], F32, tag="ls")
            nc.scalar.activation(ls[:], ssum[:], AF.Ln)
            ot = pal2.tile([P, G, D2], F32, tag="ot")
            nc.vector.tensor_tensor(out=ot[:], in0=xs[:],
                                    in1=ls[:].to_broadcast([P, G, D2]),
                                    op=OP.subtract)
            GF = B // P                       # full 128-row groups
            nc.sync.dma_start(
                out=out2[0:GF * P, :].rearrange("(g p) f -> p g f", p=P),
                in_=ot[:, 0:GF, :])
            if B % P:
                nc.sync.dma_start(out=out2[GF * P:B, :],
                                  in_=ot[:B - GF * P, GF, :])
            fctx.close()

    # Align gather queue_num with tile's DMASW lane rotation (final
    # post-scheduling order, lane = idx%8, 8 lanes): queue = idx%4 keeps
    # each lane on exactly one queue while 4 transfers overlap.
    from concourse.tile_scheduler import DMAInst
    qi = 0

    def _fix_queues(bb):
        nonlocal qi
        for inst in bb.instructions:
            if (isinstance(inst, DMAInst)
                    and inst.engine == mybir.EngineType.Pool):
                assert isinstance(inst, mybir.InstDMAGatherAnt), inst
                inst.queue_num = qi % 4
                qi += 1
            for attr in ("body_bb", "then_bb", "else_bb"):
                sub = getattr(inst, attr, None)
                if sub is not None:
                    _fix_queues(sub)

    for bb in nc.m.functions[0].blocks:
        _fix_queues(bb)

    nc.compile()
    return nc


def make_in_maps(dims: Dims, inputs: dict, per_core_meta):
    """Per-core input maps. W1/a1*/b1 columns are reordered to the
    (c,h)-interleaved layout the kernel uses internally (pure relayout)."""
    H1, C1, D1 = dims.H1, dims.C1, dims.D1
    perm = np.arange(D1).reshape(H1, C1).T.reshape(-1)   # [h*C+c] -> [c*H+h]
    x = np.asarray(inputs["x"], dtype=np.float32)
    W2 = np.asarray(inputs["W2"], np.float32)
    reps = {
        "W1": np.ascontiguousarray(
            np.asarray(inputs["W1"], np.float32)[:, perm]).astype(
                ml_dtypes.bfloat16),
        "a1s": np.ascontiguousarray(
            np.asarray(inputs["a1_src"], np.float32).reshape(-1)[perm]),
        "a1d": np.ascontiguousarray(
            np.asarray(inputs["a1_dst"], np.float32).reshape(-1)[perm]),
        "b1": np.ascontiguousarray(
            np.asarray(inputs["b1"], np.float32).reshape(-1)[perm]),
        "W2": np.ascontiguousarray(W2[perm, :]).astype(ml_dtypes.bfloat16),
        "a2s": np.asarray(inputs["a2_src"], np.float32).reshape(-1),
        "a2d": np.asarray(inputs["a2_dst"], np.float32).reshape(-1),
        "b2": np.asarray(inputs["b2"], np.float32).reshape(-1),
    }
    in_maps = []
    B = dims.B
    for k in range(dims.NC):
        m = dict(reps)
        m["xT"] = np.ascontiguousarray(
            x[k * B:(k + 1) * B, :].T).astype(ml_dtypes.bfloat16)
        m.update(per_core_meta[k])
        in_maps.append(m)
    return in_maps


_CACHE = {}


def _get_program(dims: Dims):
    key = (dims.N, dims.E, dims.NC, dims.bA, dims.bB)
    if key not in _CACHE:
        _CACHE[key] = build_program(dims)
    return _CACHE[key]


def kernel(x: np.ndarray, edge_index: np.ndarray, W1, a1_src, a1_dst, b1,
           W2, a2_src, a2_dst, b2) -> np.ndarray:
    x = np.asarray(x)
    edge_index = np.asarray(edge_index)
    dims = Dims(N=x.shape[0], E=edge_index.shape[1], n_cores=8)
    per_core = host_prep(dims, edge_index)
    nc = _get_program(dims)
    in_maps = make_in_maps(
        dims,
        dict(x=x, edge_index=edge_index, W1=W1, a1_src=a1_src, a1_dst=a1_dst,
             b1=b1, W2=W2, a2_src=a2_src, a2_dst=a2_dst, b2=b2),
        per_core)
    res = run_bass_kernel_spmd(nc, in_maps, core_ids=list(range(dims.NC)))
    out = np.concatenate([r["out2"] for r in res.results], axis=0)
    return out.astype(np.float32)



# revision 32
# speedup vs baseline: 1.0995x; 1.0232x over previous
"""2-layer GAT (PyG GATConv style) distributed across 8 TRN2 NeuronCores.

Sharding (per hint): nodes partitioned into 8 contiguous destination blocks.
Per core:
  A. xp1 = x_blk @ W1 and per-node attention logits for its own block are
     packed into a per-node gather-row table (bf16, 768B pitch,
     row = [xp1 (256, (c,h)-interleaved) | als1 (4)]); dst-side logits go to
     a 256B-pitch side table. The row table is AllGathered so every core
     holds all N rows.
  C. layer-1 edge phase over the core's own destination block. Edges are
     bucketed by 128-wide destination windows ("groups") and, within a
     group, into bins of Q=4 slots sharing one destination, laid out on
     (partition, bin-tile) cells -- so dst-side data broadcasts along the
     free dim with unit-stride APs. Source rows are fetched with per-group
     dma_gather calls (int16 indices, table split in two <32768-row
     halves; each call stays under the 128-entry SWDGE FIFO and calls
     rotate across the 4 SWDGE queues so transfers overlap); per-edge
     softmax numerators ex = exp(leaky_relu(als[s]+ald[d])) are computed
     on DVE/ACT (leaky-relu scale and all PSUM reads ride the Scalar
     engine), and messages are segment-summed by host-precomputed 0/1
     selector matmuls on the TensorEngine (PSUM accumulation per group).
     The softmax normalizer rides as extra rhs columns of the same
     matmul, so alpha normalization is a per-node epilogue. ELU and the
     layer-2 node table are fused into the epilogue; that table is
     AllGathered.
  F. layer-2 edge phase (same structure, 17-wide rows); log_softmax is a
     single deferred pass over all groups at the end.

Host preprocessing is index-only (self-loops, bucketing, padding,
selector one-hots, relayout); all float math runs on device.
"""

import math
from contextlib import ExitStack

import numpy as np
import ml_dtypes

import concourse.bass as bass
import concourse.tile as tile
from concourse import bacc, mybir
from concourse.bass_utils import run_bass_kernel_spmd
from concourse.masks import make_identity

F32 = mybir.dt.float32
BF16 = mybir.dt.bfloat16
F8 = mybir.dt.float8e4
I16 = mybir.dt.int16
AF = mybir.ActivationFunctionType
OP = mybir.AluOpType

P = 128
Q = 4                 # slots per bin (same-dst edges per partition-cell)
NEG_SLOPE = 0.2


_GQ = {"i": 0}


def _next_q(nq=4):
    q = _GQ["i"] % nq
    _GQ["i"] += 1
    return q


def dma_gather_raw(gp, out_ap, in_ap, idxs_ap, num_idxs, elem_size,
                   elem_step, single_packet=None, queue_num=None):
    """BassGpSimd.dma_gather minus the payload%256 assert (pitch must still
    be a 256B multiple; verified on HW that arbitrary payload works).

    num_idxs must stay <= ~2032: the per-queue SWDGE FIFO holds 128
    entries and a call consumes num_idxs/16+1. Rotating queue_num lets
    transfers from consecutive calls overlap (one outstanding per queue).
    """
    from concourse._compat import exact_div
    if single_packet is None:
        # single-packet mode breaks somewhere between 1024 and 1536 indices
        single_packet = num_idxs <= 1024
    if queue_num is None:
        queue_num = _next_q()
    assert num_idxs <= 2032, num_idxs
    assert idxs_ap.dtype == mybir.dt.int16
    assert in_ap.dtype == out_ap.dtype
    stride_bytes = elem_step * mybir.dt.size(in_ap.dtype)
    stride_bytes_256 = exact_div(stride_bytes, 256)
    assert stride_bytes_256 < 256
    _in_ap = gp.lower_ap_dma(in_ap, for_custom_bir_dma=True)
    _idxs_ap = gp.lower_ap(idxs_ap)
    _out_ap = gp.lower_ap(out_ap)
    return gp.add_instruction(
        mybir.InstDMAGatherAnt(
            name=gp.bass.get_next_instruction_name(),
            ins=[*_in_ap, _idxs_ap, gp.lower_val_access(gp.to_reg(num_idxs))],
            outs=[_out_ap],
            transpose=False,
            num_idxs=num_idxs,
            elem_size=elem_size,
            stride_bytes_256=stride_bytes_256,
            gen_mode=0,
            single_packet=single_packet,
            queue_num=queue_num,
            sbuf_tokens_per_rank=0,
            sbuf_free_dim_per_rank=0,
            sbuf_free_dim_pad_per_rank=0,
            sbuf_byte_offset=0,
        ))


class Dims:
    def __init__(self, N, E, n_cores, H1=4, C1=64, H2=1, C2=16, F_in=256):
        self.N, self.E, self.NC = N, E, n_cores
        self.F_in = F_in
        self.H1, self.C1, self.H2, self.C2 = H1, C1, H2, C2
        self.D1 = H1 * C1
        self.D2 = H2 * C2
        self.B = N // n_cores
        self.G = math.ceil(self.B / P)
        self.HALF = min(25000, (N + 1) // 2)  # int16 table split point
        self.R1 = self.D1 + self.H1          # gathered row 1: xp1|als1
        self.T1 = 384                         # table-1 pitch (768B bf16)
        self.R2 = self.D2 + self.H2          # gathered row 2: xp2|als2
        self.T2 = 128                         # table-2 pitch (256B bf16)
        self.TA = 128                         # ald side-table pitch (256B)
        self.bA = None   # bin-tiles for src-half A (set by host_prep)
        self.bB = None
        self.KB = None   # bA + bB
        self.COLS = None  # KB * Q edge columns per group


def _wrap_idx16(flat):
    """index list -> [128, ceil(n/16)] int16 SBUF image (16-partition wrap,
    replicated for the 8 Q7 cores)."""
    n = len(flat)
    S = math.ceil(n / 16)
    a = np.zeros((16, S), np.int16)
    i = np.arange(n)
    a[i % 16, i // 16] = flat
    return np.tile(a, (8, 1))


def host_prep(dims: Dims, edge_index: np.ndarray):
    """Index-only preprocessing: self-loops, per-core dst blocks, 128-dst
    windows, same-dst bins of Q slots split by src half, padding, int16
    index images."""
    N, NC, B, G = dims.N, dims.NC, dims.B, dims.G
    HALF = dims.HALF
    loops = np.arange(N, dtype=np.int64)
    src = np.concatenate([edge_index[0].astype(np.int64), loops])
    dst = np.concatenate([edge_index[1].astype(np.int64), loops])

    # per (core, group, half): list of (dst_local, [srcs]) bins
    cores = []
    bmaxA = bmaxB = 1
    for k in range(NC):
        lo, hi = k * B, (k + 1) * B
        m = (dst >= lo) & (dst < hi)
        s_k = src[m]
        d_k = dst[m] - lo
        order = np.lexsort((s_k >= HALF, d_k))
        s_k, d_k = s_k[order], d_k[order]
        half_k = (s_k >= HALF).astype(np.int64)
        groups = []
        for g in range(G):
            gmask = (d_k // P) == g
            sg, dg, hg = s_k[gmask], d_k[gmask] - g * P, half_k[gmask]
            binsA, binsB = [], []
            for h, bins in ((0, binsA), (1, binsB)):
                hm = hg == h
                sh, dh = sg[hm], dg[hm]
                # consecutive same-dst runs -> bins of <= Q edges
                ptr = 0
                n = len(sh)
                while ptr < n:
                    d0 = dh[ptr]
                    end = ptr
                    while end < n and dh[end] == d0 and end - ptr < Q:
                        end += 1
                    bins.append((int(d0), sh[ptr:end]))
                    ptr = end
            groups.append((binsA, binsB))
            bmaxA = max(bmaxA, math.ceil(len(binsA) / P))
            bmaxB = max(bmaxB, math.ceil(len(binsB) / P))
        cores.append(groups)
    dims.bA, dims.bB = bmaxA, bmaxB
    dims.KB = bmaxA + bmaxB
    dims.COLS = dims.KB * Q

    G_, KB, COLS = G, dims.KB, dims.COLS
    per_core = []
    for k in range(NC):
        # slot-level arrays
        srcl = np.zeros((P, G_, COLS), np.int64)   # half-local src id
        emask = np.full((P, G_, COLS), -150.0, np.float32)  # pad-slot mask
        dstl = np.full((P, G_, KB), -1.0, np.float32)  # per-bin local dst
        binid = np.zeros((P, G_, KB), np.int64)    # per-bin LOCAL dst row id
        for g in range(G_):
            binsA, binsB = cores[k][g]
            for hoff, bins in ((0, binsA), (dims.bA, binsB)):
                for b, (d0, ss) in enumerate(bins):
                    p = b % P
                    bt = hoff + b // P
                    dstl[p, g, bt] = float(d0)
                    binid[p, g, bt] = g * P + d0
                    for q, s in enumerate(ss):
                        srcl[p, g, bt * Q + q] = s if s < HALF else s - HALF
                        emask[p, g, bt * Q + q] = 0.0
        # int16 gather-index images per group, concatenated along free dim
        SA = dims.bA * Q * P // 16            # idxA image cols per group
        SB = dims.bB * Q * P // 16
        SBN = KB * P // 16
        idxA = np.zeros((P, G_ * SA), np.int16)
        idxB = np.zeros((P, G_ * SB), np.int16)
        idxN = np.zeros((P, G_ * SBN), np.int16)
        for g in range(G_):
            cA = dims.bA * Q
            flatA = srcl[:, g, 0:cA].T.reshape(-1)      # i = c*128 + p
            flatB = srcl[:, g, cA:COLS].T.reshape(-1)
            flatN = binid[:, g, :].T.reshape(-1)
            idxA[:, g * SA:(g + 1) * SA] = _wrap_idx16(flatA)
            idxB[:, g * SB:(g + 1) * SB] = _wrap_idx16(flatB)
            idxN[:, g * SBN:(g + 1) * SBN] = _wrap_idx16(flatN)
        # selector one-hots: stsel[p, (g,bt,d)] = 1 iff dstl[p,g,bt] == d
        stsel = np.zeros((P, G_, KB, P), ml_dtypes.float8_e4m3)
        pp, gg, bb = np.nonzero(dstl >= 0)
        stsel[pp, gg, bb, dstl[pp, gg, bb].astype(np.int64)] = 1.0
        per_core.append(dict(
            idxA=idxA, idxB=idxB, idxN=idxN,
            stsel=np.ascontiguousarray(stsel.reshape(P, G_ * KB * P)),
            dstl=np.ascontiguousarray(
                dstl.reshape(P, G_ * KB)).astype(ml_dtypes.bfloat16),
            emask=np.ascontiguousarray(
                emask.reshape(P, G_ * COLS)).astype(ml_dtypes.bfloat16),
        ))
    return per_core


def build_program(dims: Dims, replicas: int = 1, stages: str = "agchf"):
    N, NC, B, G = dims.N, dims.NC, dims.B, dims.G
    F_in, D1, D2, H1, H2 = dims.F_in, dims.D1, dims.D2, dims.H1, dims.H2
    C1 = dims.C1
    R1, T1, R2, T2, TA = dims.R1, dims.T1, dims.R2, dims.T2, dims.TA
    bAt, bBt, KB, COLS = dims.bA, dims.bB, dims.KB, dims.COLS
    KF = F_in // P
    KD = D1 // P
    SA = bAt * Q * P // 16
    SB = bBt * Q * P // 16
    SBN = KB * P // 16
    nA = bAt * Q * P     # indices per gather call A
    nB = bBt * Q * P
    nN = KB * P
    HALF = dims.HALF

    _GQ["i"] = 0
    nc = bacc.Bacc("TRN2", target_bir_lowering=False, debug=False,
                   enable_asserts=False, num_devices=NC,
                   num_swdge_queues=4)

    xT = nc.dram_tensor("xT", [F_in, B], BF16, kind="ExternalInput")
    W1 = nc.dram_tensor("W1", [F_in, D1], BF16, kind="ExternalInput")
    a1s = nc.dram_tensor("a1s", [D1], F32, kind="ExternalInput")
    a1d = nc.dram_tensor("a1d", [D1], F32, kind="ExternalInput")
    b1 = nc.dram_tensor("b1", [D1], F32, kind="ExternalInput")
    W2 = nc.dram_tensor("W2", [D1, D2], BF16, kind="ExternalInput")
    a2s = nc.dram_tensor("a2s", [D2], F32, kind="ExternalInput")
    a2d = nc.dram_tensor("a2d", [D2], F32, kind="ExternalInput")
    b2 = nc.dram_tensor("b2", [D2], F32, kind="ExternalInput")
    idxA = nc.dram_tensor("idxA", [P, G * SA], I16, kind="ExternalInput")
    idxB = nc.dram_tensor("idxB", [P, G * SB], I16, kind="ExternalInput")
    idxN = nc.dram_tensor("idxN", [P, G * SBN], I16, kind="ExternalInput")
    stsel = nc.dram_tensor("stsel", [P, G * KB * P], F8,
                           kind="ExternalInput")
    dstl = nc.dram_tensor("dstl", [P, G * KB], BF16, kind="ExternalInput")
    emask = nc.dram_tensor("emask", [P, G * COLS], BF16, kind="ExternalInput")
    out2 = nc.dram_tensor("out2", [B, D2], F32, kind="ExternalOutput")

    t1_loc = nc.dram_tensor("t1_loc", [B, T1], BF16)
    t1_full = nc.dram_tensor("t1_full", [N, T1], BF16, addr_space="Shared")
    ald1t = nc.dram_tensor("ald1t", [B, TA], BF16)
    t2_loc = nc.dram_tensor("t2_loc", [B, T2], BF16)
    t2_full = nc.dram_tensor("t2_full", [N, T2], BF16, addr_space="Shared")

    rg = [list(range(NC))]

    with tile.TileContext(nc) as tc, ExitStack() as ctx:
        const = ctx.enter_context(tc.tile_pool(name="const", bufs=1))
        ictx = ExitStack()
        cpsum = ictx.enter_context(tc.tile_pool(name="cpsum", bufs=1,
                                                space="PSUM"))

        iota_i = const.tile([P, P], mybir.dt.int32, tag="iota_i")
        nc.gpsimd.iota(iota_i[:], pattern=[[1, P]], base=0,
                       channel_multiplier=0)
        iota_bf = const.tile([P, P], BF16, tag="iota_bf")
        nc.vector.tensor_copy(iota_bf[:], iota_i[:])
        ident = const.tile([P, P], BF16, tag="ident")
        make_identity(nc, ident[:])

        w1sb = const.tile([P, KF, D1], BF16, tag="w1sb")
        for c in range(KF):
            nc.sync.dma_start(out=w1sb[:, c, :], in_=W1[c * P:(c + 1) * P, :])
        w2sb = const.tile([P, KD, D2], BF16, tag="w2sb")
        for c in range(KD):
            nc.sync.dma_start(out=w2sb[:, c, :], in_=W2[c * P:(c + 1) * P, :])

        ones_row = const.tile([1, P], F32, tag="ones_row")
        nc.vector.memset(ones_row[:], 1.0)

        def replicate(vec_ap, X, tag):
            vrow = const.tile([1, X], F32, tag=tag + "_row")
            nc.sync.dma_start(out=vrow[:], in_=vec_ap[None, :])
            pr = cpsum.tile([P, X], F32, tag="reppsum")
            nc.tensor.matmul(out=pr[:], lhsT=ones_row[:], rhs=vrow[:],
                             start=True, stop=True)
            rep = const.tile([P, X], F32, tag=tag)
            nc.vector.tensor_copy(rep[:], pr[:])
            return rep

        a1s_r = replicate(a1s, D1, "a1s_r")
        a1d_r = replicate(a1d, D1, "a1d_r")
        b1_r = replicate(b1, D1, "b1_r")
        a2s_r = replicate(a2s, D2, "a2s_r")
        a2d_r = replicate(a2d, D2, "a2d_r")
        b2_r = replicate(b2, D2, "b2_r")

        idxA_sb = const.tile([P, G * SA], I16, tag="idxA_sb")
        nc.sync.dma_start(out=idxA_sb[:], in_=idxA[:, :])
        idxB_sb = const.tile([P, G * SB], I16, tag="idxB_sb")
        nc.sync.dma_start(out=idxB_sb[:], in_=idxB[:, :])
        idxN_sb = const.tile([P, G * SBN], I16, tag="idxN_sb")
        nc.sync.dma_start(out=idxN_sb[:], in_=idxN[:, :])
        dstl_sb = const.tile([P, G * KB], BF16, tag="dstl_sb")
        nc.sync.dma_start(out=dstl_sb[:], in_=dstl[:, :])
        emask_sb = const.tile([P, G * COLS], BF16, tag="emask_sb")
        nc.sync.dma_start(out=emask_sb[:], in_=emask[:, :])

        ictx.close()

        for _rep in range(replicas):
          if "a" in stages:
            # ---- stage A: layer-1 node table for own block -----------------
            actx = ExitStack()
            pa = actx.enter_context(tc.tile_pool(name="pa", bufs=3))
            prowa = actx.enter_context(tc.tile_pool(name="prowa", bufs=10))
            pa_ps = actx.enter_context(tc.tile_pool(name="pa_ps", bufs=2,
                                                    space="PSUM"))
            for t in range(G):
                n0 = t * P
                nn = min(P, B - n0)
                xta = pa.tile([P, KF, P], BF16, tag="xta")
                for c in range(KF):
                    nc.sync.dma_start(out=xta[:, c, :nn],
                                      in_=xT[c * P:(c + 1) * P, n0:n0 + nn])
                ps_xp = pa_ps.tile([P, D1], F32, tag="ps_xp")
                for c in range(KF):
                    nc.tensor.matmul(out=ps_xp[:nn, :], lhsT=xta[:, c, :nn],
                                     rhs=w1sb[:, c, :],
                                     start=(c == 0), stop=(c == KF - 1))
                tmp = pa.tile([P, D1], F32, tag="tmpa")
                alf = pa.tile([P, 2 * H1], F32, tag="alf")
                row = prowa.tile([P, R1], BF16, tag="row1")
                rowd = prowa.tile([P, H1], BF16, tag="rowd")
                nc.vector.tensor_tensor(out=tmp[:nn], in0=ps_xp[:nn],
                                        in1=a1s_r[:nn], op=OP.mult)
                # features are (c,h)-interleaved: head h at stride-4 positions
                nc.vector.tensor_reduce(
                    out=alf[:nn, 0:H1],
                    in_=tmp[:nn].rearrange("p (c h) -> p h c", h=H1),
                    axis=mybir.AxisListType.X, op=OP.add)
                nc.vector.tensor_tensor(out=tmp[:nn], in0=ps_xp[:nn],
                                        in1=a1d_r[:nn], op=OP.mult)
                nc.vector.tensor_reduce(
                    out=alf[:nn, H1:2 * H1],
                    in_=tmp[:nn].rearrange("p (c h) -> p h c", h=H1),
                    axis=mybir.AxisListType.X, op=OP.add)
                nc.vector.tensor_copy(row[:nn, D1:D1 + H1], alf[:nn, 0:H1])
                nc.vector.tensor_copy(row[:nn, 0:D1], ps_xp[:nn])
                nc.vector.tensor_copy(rowd[:nn, :], alf[:nn, H1:2 * H1])
                nc.sync.dma_start(out=t1_loc[n0:n0 + nn, 0:R1], in_=row[:nn, :])
                nc.sync.dma_start(out=ald1t[n0:n0 + nn, 0:H1], in_=rowd[:nn, :])
            actx.close()

          if "g" in stages:
            # ---- AllGather layer-1 table -----------------------------------
            nc.gpsimd.collective_compute(
                "AllGather", OP.bypass, replica_groups=rg,
                ins=[t1_loc.ap()], outs=[t1_full.ap()])

          if "c" in stages:
            # ---- stage C: layer-1 edge phase + fused layer-2 table ---------
            cctx = ExitStack()
            pal = cctx.enter_context(tc.tile_pool(name="pal", bufs=6))
            pg = cctx.enter_context(tc.tile_pool(name="pg", bufs=4))
            pm = cctx.enter_context(tc.tile_pool(name="pm", bufs=3))
            pe = cctx.enter_context(tc.tile_pool(name="pe", bufs=3))
            prow2 = cctx.enter_context(tc.tile_pool(name="prow2", bufs=12))
            pc_ps = cctx.enter_context(tc.tile_pool(name="pc_ps", bufs=3,
                                                    space="PSUM"))
            pt_ps = cctx.enter_context(tc.tile_pool(name="pt_ps", bufs=2,
                                                    space="PSUM"))
            cA = bAt * Q
            cB = bBt * Q
            # per-bin dst logits, two groups per call (<=2032-idx FIFO cap)
            for b0 in range(0, G, 2):
              gz = min(2, G - b0)
              aldbp = pal.tile([P, 2 * KB, H1], BF16, tag="aldbp")
              dma_gather_raw(nc.gpsimd, aldbp[:, 0:gz * KB, :],
                             ald1t[:, 0:H1],
                             idxN_sb[:, b0 * SBN:(b0 + gz) * SBN],
                             gz * nN, H1, TA)
              for g in range(b0, b0 + gz):
                w0 = g * P
                wn = min(P, B - w0)
                gA = pg.tile([P, cA, R1], BF16, tag="gatA")
                dma_gather_raw(nc.gpsimd, gA[:, :, :],
                               t1_full[0:HALF, 0:R1],
                               idxA_sb[:, g * SA:(g + 1) * SA], nA, R1, T1)
                gB = pg.tile([P, cB, R1], BF16, tag="gatB")
                dma_gather_raw(nc.gpsimd, gB[:, :, :],
                               t1_full[HALF:N, 0:R1],
                               idxB_sb[:, g * SB:(g + 1) * SB], nB, R1, T1)
                aldb = aldbp[:, (g - b0) * KB:(g - b0 + 1) * KB, :]

                # ex = exp(leaky_relu(als[s] + ald[d]))
                ep = pe.tile([P, KB, Q, H1], F32, tag="ep")
                nc.vector.tensor_tensor(
                    out=ep[:, 0:bAt],
                    in0=gA[:, :, D1:D1 + H1].rearrange(
                        "p (b q) h -> p b q h", q=Q),
                    in1=aldb[:, 0:bAt, None, :].to_broadcast([P, bAt, Q, H1]),
                    op=OP.add)
                nc.vector.tensor_tensor(
                    out=ep[:, bAt:KB],
                    in0=gB[:, :, D1:D1 + H1].rearrange(
                        "p (b q) h -> p b q h", q=Q),
                    in1=aldb[:, bAt:KB, None, :].to_broadcast([P, bBt, Q, H1]),
                    op=OP.add)
                lr = pe.tile([P, KB, Q, H1], F32, tag="lr")
                nc.scalar.activation(lr[:], ep[:], AF.Copy, scale=NEG_SLOPE)
                nc.vector.tensor_tensor(out=lr[:], in0=lr[:], in1=ep[:],
                                        op=OP.max)
                nc.vector.tensor_tensor(
                    out=lr[:], in0=lr[:],
                    in1=emask_sb[:, g * COLS:(g + 1) * COLS].rearrange(
                        "p (b q) -> p b q", q=Q)[:, :, :, None].to_broadcast(
                        [P, KB, Q, H1]),
                    op=OP.add)
                msg = pm.tile([P, COLS, D1 + H1], BF16, tag="msg")
                nc.scalar.activation(
                    msg[:, :, D1:D1 + H1].rearrange("p (b q) h -> p b q h", q=Q),
                    lr[:], AF.Exp)
                # msg features: ex broadcast over c with unit-stride inner h
                nc.vector.tensor_tensor(
                    out=msg[:, 0:cA, 0:D1].rearrange(
                        "p k (c h) -> p k c h", h=H1),
                    in0=gA[:, :, 0:D1].rearrange("p k (c h) -> p k c h", h=H1),
                    in1=msg[:, 0:cA, D1:D1 + H1][:, :, None, :].to_broadcast(
                        [P, cA, C1, H1]),
                    op=OP.mult)
                nc.vector.tensor_tensor(
                    out=msg[:, cA:COLS, 0:D1].rearrange(
                        "p k (c h) -> p k c h", h=H1),
                    in0=gB[:, :, 0:D1].rearrange("p k (c h) -> p k c h", h=H1),
                    in1=msg[:, cA:COLS, D1:D1 + H1][:, :, None, :].to_broadcast(
                        [P, cB, C1, H1]),
                    op=OP.mult)

                st = pm.tile([P, KB, P], F8, tag="st")
                nc.sync.dma_start(out=st[:],
                                  in_=stsel[:, g * KB * P:(g + 1) * KB * P])

                ps_g = pc_ps.tile([P, D1 + H1], F32, tag="ps_g")
                for col in range(COLS):
                    nc.tensor.matmul(out=ps_g[:], lhsT=st[:, col // Q, :],
                                     rhs=msg[:, col, :],
                                     start=(col == 0), stop=(col == COLS - 1))

                # epilogue: alpha-normalize, +b1, ELU -> h1 (bf16).
                # PSUM leaves via ACT only (DVE PSUM reads stall under
                # concurrent PE writes).
                hps = pe.tile([P, D1 + H1], F32, tag="hps")
                nc.scalar.activation(hps[:wn], ps_g[:wn], AF.Copy)
                rec = pe.tile([P, H1], F32, tag="rec")
                nc.vector.reciprocal(rec[:wn], hps[:wn, D1:D1 + H1])
                h1f = pg.tile([P, D1], F32, tag="h1f")
                nc.vector.tensor_tensor(
                    out=h1f[:wn].rearrange("p (c h) -> p c h", h=H1),
                    in0=hps[:wn, 0:D1].rearrange("p (c h) -> p c h", h=H1),
                    in1=rec[:wn][:, None, :].to_broadcast([wn, C1, H1]),
                    op=OP.mult)
                nc.vector.tensor_tensor(out=h1f[:wn], in0=h1f[:wn], in1=b1_r[:wn],
                                        op=OP.add)
                # ELU: relu(x) + exp(-relu(-x)) - 1, relus+exp on ACT
                mn = pe.tile([P, D1], F32, tag="mn")
                nc.scalar.activation(mn[:wn], h1f[:wn], AF.Relu, scale=-1.0)
                em = pe.tile([P, D1], F32, tag="em")
                nc.scalar.activation(em[:wn], mn[:wn], AF.Exp, scale=-1.0)
                rl = pe.tile([P, D1], F32, tag="rl")
                nc.scalar.activation(rl[:wn], h1f[:wn], AF.Relu)
                nc.vector.tensor_scalar_add(em[:wn], em[:wn], -1.0)
                h1b = pg.tile([P, D1], BF16, tag="h1b")
                nc.vector.tensor_tensor(out=h1b[:wn], in0=rl[:wn], in1=em[:wn],
                                        op=OP.add)

                # fused layer-2 node-table build
                row2 = prow2.tile([P, 2 * H2], BF16, tag="row2")
                ps_x2 = pt_ps.tile([P, D2], F32, tag="ps_x2")
                for c in range(KD):
                    pt = pt_ps.tile([P, P], BF16, tag="pt")
                    nc.tensor.transpose(pt[:], h1b[:, c * P:(c + 1) * P], ident[:])
                    cpt = pe.tile([P, P], BF16, tag="cpt")
                    nc.vector.tensor_copy(cpt[:], pt[:])
                    nc.tensor.matmul(out=ps_x2[:], lhsT=cpt[:], rhs=w2sb[:, c, :],
                                     start=(c == 0), stop=(c == KD - 1))
                x2fb = prow2.tile([P, D2], BF16, tag="x2fb")
                nc.scalar.activation(x2fb[:wn], ps_x2[:wn], AF.Copy)
                t2m = pe.tile([P, D2], F32, tag="t2m")
                nc.vector.tensor_tensor(out=t2m[:wn], in0=x2fb[:wn],
                                        in1=a2s_r[:wn], op=OP.mult)
                with nc.allow_low_precision(reason="16-wide sum to bf16"):
                    nc.vector.tensor_reduce(out=row2[:wn, 0:1], in_=t2m[:wn],
                                            axis=mybir.AxisListType.X,
                                            op=OP.add)
                nc.vector.tensor_tensor(out=t2m[:wn], in0=x2fb[:wn],
                                        in1=a2d_r[:wn], op=OP.mult)
                with nc.allow_low_precision(reason="16-wide sum to bf16"):
                    nc.vector.tensor_reduce(out=row2[:wn, 1:2], in_=t2m[:wn],
                                            axis=mybir.AxisListType.X,
                                            op=OP.add)
                # t2_loc row = [xp2 | als2 | ald2]; ald2 stays local-only
                nc.sync.dma_start(out=t2_loc[w0:w0 + wn, 0:D2],
                                  in_=x2fb[:wn, :])
                nc.sync.dma_start(out=t2_loc[w0:w0 + wn, D2:D2 + 2 * H2],
                                  in_=row2[:wn, :])
            cctx.close()

          if "h" in stages:
            # ---- AllGather layer-2 table -----------------------------------
            nc.gpsimd.collective_compute(
                "AllGather", OP.bypass, replica_groups=rg,
                ins=[t2_loc.ap()], outs=[t2_full.ap()])

          if "f" in stages:
            # ---- stage F: layer-2 edge phase + log_softmax ------------------
            fctx = ExitStack()
            pal2 = fctx.enter_context(tc.tile_pool(name="pal2", bufs=1))
            paf = fctx.enter_context(tc.tile_pool(name="paf", bufs=6))
            pf = fctx.enter_context(tc.tile_pool(name="pf", bufs=4))
            pfg = fctx.enter_context(tc.tile_pool(name="pfg", bufs=6))
            pf_ps = fctx.enter_context(tc.tile_pool(name="pf_ps", bufs=4,
                                                    space="PSUM"))
            cA = bAt * Q
            cB = bBt * Q
            x2_all = pal2.tile([P, G, D2], F32, tag="x2_all")
            nc.vector.memset(x2_all[:], 0.0)   # last group's tail rows
            for b0 in range(0, G, 2):
              gz = min(2, G - b0)
              ald2p = paf.tile([P, 2 * KB, H2], BF16, tag="ald2p")
              dma_gather_raw(nc.gpsimd, ald2p[:, 0:gz * KB, :],
                             t2_loc[:, R2:R2 + H2],
                             idxN_sb[:, b0 * SBN:(b0 + gz) * SBN],
                             gz * nN, H2, T2)
              for g in range(b0, b0 + gz):
                w0 = g * P
                wn = min(P, B - w0)
                g2A = pfg.tile([P, cA, R2], BF16, tag="gat2A")
                dma_gather_raw(nc.gpsimd, g2A[:, :, :],
                               t2_full[0:HALF, 0:R2],
                               idxA_sb[:, g * SA:(g + 1) * SA], nA, R2, T2)
                g2B = pfg.tile([P, cB, R2], BF16, tag="gat2B")
                dma_gather_raw(nc.gpsimd, g2B[:, :, :],
                               t2_full[HALF:N, 0:R2],
                               idxB_sb[:, g * SB:(g + 1) * SB], nB, R2, T2)
                ald2b = ald2p[:, (g - b0) * KB:(g - b0 + 1) * KB, :]

                ep2 = pf.tile([P, KB, Q, H2], F32, tag="ep2")
                nc.vector.tensor_tensor(
                    out=ep2[:, 0:bAt],
                    in0=g2A[:, :, D2:D2 + H2].rearrange(
                        "p (b q) h -> p b q h", q=Q),
                    in1=ald2b[:, 0:bAt, None, :].to_broadcast([P, bAt, Q, H2]),
                    op=OP.add)
                nc.vector.tensor_tensor(
                    out=ep2[:, bAt:KB],
                    in0=g2B[:, :, D2:D2 + H2].rearrange(
                        "p (b q) h -> p b q h", q=Q),
                    in1=ald2b[:, bAt:KB, None, :].to_broadcast([P, bBt, Q, H2]),
                    op=OP.add)
                lr2 = pf.tile([P, KB, Q, H2], F32, tag="lr2")
                nc.scalar.activation(lr2[:], ep2[:], AF.Copy, scale=NEG_SLOPE)
                nc.vector.tensor_tensor(out=lr2[:], in0=lr2[:], in1=ep2[:],
                                        op=OP.max)
                nc.vector.tensor_tensor(
                    out=lr2[:], in0=lr2[:],
                    in1=emask_sb[:, g * COLS:(g + 1) * COLS].rearrange(
                        "p (b q) -> p b q", q=Q)[:, :, :, None].to_broadcast(
                        [P, KB, Q, H2]),
                    op=OP.add)
                msg2 = pf.tile([P, COLS, R2], BF16, tag="msg2")
                nc.scalar.activation(
                    msg2[:, :, D2:D2 + H2].rearrange("p (b q) h -> p b q h", q=Q),
                    lr2[:], AF.Exp)
                nc.vector.tensor_tensor(
                    out=msg2[:, 0:cA, 0:D2],
                    in0=g2A[:, :, 0:D2],
                    in1=msg2[:, 0:cA, D2:D2 + H2].to_broadcast([P, cA, D2]),
                    op=OP.mult)
                nc.vector.tensor_tensor(
                    out=msg2[:, cA:COLS, 0:D2],
                    in0=g2B[:, :, 0:D2],
                    in1=msg2[:, cA:COLS, D2:D2 + H2].to_broadcast([P, cB, D2]),
                    op=OP.mult)

                st2 = pf.tile([P, KB, P], BF16, tag="st2")
                nc.vector.tensor_tensor(
                    out=st2[:],
                    in0=iota_bf[:, None, :].to_broadcast([P, KB, P]),
                    in1=dstl_sb[:, g * KB:(g + 1) * KB][:, :, None
                        ].to_broadcast([P, KB, P]),
                    op=OP.is_equal)

                ps2 = pf_ps.tile([P, R2], F32, tag="ps2")
                for col in range(COLS):
                    nc.tensor.matmul(out=ps2[:], lhsT=st2[:, col // Q, :],
                                     rhs=msg2[:, col, :],
                                     start=(col == 0), stop=(col == COLS - 1))

                rec2 = pf.tile([P, H2], F32, tag="rec2")
                nc.vector.reciprocal(rec2[:wn], ps2[:wn, D2:D2 + H2])
                x2 = pf.tile([P, D2], F32, tag="x2")
                nc.scalar.activation(x2[:wn], ps2[:wn, 0:D2], AF.Copy,
                                     scale=rec2[:wn])
                nc.vector.tensor_tensor(out=x2_all[:wn, g, :], in0=x2[:wn],
                                        in1=b2_r[:wn], op=OP.add)

            # deferred log_softmax over all groups (one Exp + one Ln pass)
            mx = pal2.tile([P, G, 1], F32, tag="mx")
            nc.vector.tensor_reduce(out=mx[:], in_=x2_all[:],
                                    axis=mybir.AxisListType.X, op=OP.max)
            xs = pal2.tile([P, G, D2], F32, tag="xs")
            nc.vector.tensor_tensor(out=xs[:], in0=x2_all[:],
                                    in1=mx[:].to_broadcast([P, G, D2]),
                                    op=OP.subtract)
            es = pal2.tile([P, G, D2], F32, tag="es")
            nc.scalar.activation(es[:], xs[:], AF.Exp)
            ssum = pal2.tile([P, G, 1], F32, tag="ssum")
            nc.vector.tensor_reduce(out=ssum[:], in_=es[:],
                                    axis=mybir.AxisListType.X, op=OP.add)
            ls = pal2.tile([P, G, 1], F32, tag="ls")
            nc.scalar.activation(ls[:], ssum[:], AF.Ln)
            ot = pal2.tile([P, G, D2], F32, tag="ot")
            nc.vector.tensor_tensor(out=ot[:], in0=xs[:],
                                    in1=ls[:].to_broadcast([P, G, D2]),
                                    op=OP.subtract)
            GF = B // P                       # full 128-row groups
            nc.sync.dma_start(
                out=out2[0:GF * P, :].rearrange("(g p) f -> p g f", p=P),
                in_=ot[:, 0:GF, :])
            if B % P:
                nc.sync.dma_start(out=out2[GF * P:B, :],
                                  in_=ot[:B - GF * P, GF, :])
            fctx.close()

    # Align gather queue_num with tile's DMASW lane rotation (final
    # post-scheduling order, lane = idx%8, 8 lanes): queue = idx%4 keeps
    # each lane on exactly one queue while 4 transfers overlap.
    from concourse.tile_scheduler import DMAInst
    qi = 0

    def _fix_queues(bb):
        nonlocal qi
        for inst in bb.instructions:
            if (isinstance(inst, DMAInst)
                    and inst.engine == mybir.EngineType.Pool):
                assert isinstance(inst, mybir.InstDMAGatherAnt), inst
                inst.queue_num = qi % 4
                qi += 1
            for attr in ("body_bb", "then_bb", "else_bb"):
                sub = getattr(inst, attr, None)
                if sub is not None:
                    _fix_queues(sub)

    for bb in nc.m.functions[0].blocks:
        _fix_queues(bb)

    nc.compile()
    return nc


def make_in_maps(dims: Dims, inputs: dict, per_core_meta):
    """Per-core input maps. W1/a1*/b1 columns are reordered to the
    (c,h)-interleaved layout the kernel uses internally (pure relayout)."""
    H1, C1, D1 = dims.H1, dims.C1, dims.D1
    perm = np.arange(D1).reshape(H1, C1).T.reshape(-1)   # [h*C+c] -> [c*H+h]
    x = np.asarray(inputs["x"], dtype=np.float32)
    W2 = np.asarray(inputs["W2"], np.float32)
    reps = {
        "W1": np.ascontiguousarray(
            np.asarray(inputs["W1"], np.float32)[:, perm]).astype(
                ml_dtypes.bfloat16),
        "a1s": np.ascontiguousarray(
            np.asarray(inputs["a1_src"], np.float32).reshape(-1)[perm]),
        "a1d": np.ascontiguousarray(
            np.asarray(inputs["a1_dst"], np.float32).reshape(-1)[perm]),
        "b1": np.ascontiguousarray(
            np.asarray(inputs["b1"], np.float32).reshape(-1)[perm]),
        "W2": np.ascontiguousarray(W2[perm, :]).astype(ml_dtypes.bfloat16),
        "a2s": np.asarray(inputs["a2_src"], np.float32).reshape(-1),
        "a2d": np.asarray(inputs["a2_dst"], np.float32).reshape(-1),
        "b2": np.asarray(inputs["b2"], np.float32).reshape(-1),
    }
    in_maps = []
    B = dims.B
    for k in range(dims.NC):
        m = dict(reps)
        m["xT"] = np.ascontiguousarray(
            x[k * B:(k + 1) * B, :].T).astype(ml_dtypes.bfloat16)
        m.update(per_core_meta[k])
        in_maps.append(m)
    return in_maps


_CACHE = {}


def _get_program(dims: Dims):
    key = (dims.N, dims.E, dims.NC, dims.bA, dims.bB)
    if key not in _CACHE:
        _CACHE[key] = build_program(dims)
    return _CACHE[key]


def kernel(x: np.ndarray, edge_index: np.ndarray, W1, a1_src, a1_dst, b1,
           W2, a2_src, a2_dst, b2) -> np.ndarray:
    x = np.asarray(x)
    edge_index = np.asarray(edge_index)
    dims = Dims(N=x.shape[0], E=edge_index.shape[1], n_cores=8)
    per_core = host_prep(dims, edge_index)
    nc = _get_program(dims)
    in_maps = make_in_maps(
        dims,
        dict(x=x, edge_index=edge_index, W1=W1, a1_src=a1_src, a1_dst=a1_dst,
             b1=b1, W2=W2, a2_src=a2_src, a2_dst=a2_dst, b2=b2),
        per_core)
    res = run_bass_kernel_spmd(nc, in_maps, core_ids=list(range(dims.NC)))
    out = np.concatenate([r["out2"] for r in res.results], axis=0)
    return out.astype(np.float32)

